# revision 2
# baseline (speedup 1.0000x reference)
"""Trainium2 Bass kernel for nn_CFDFVGCN (two SpatialGraphConv layers, concat).

Strategy (8 NeuronCores, SPMD single program):
  - Shard by DESTINATION node windows of 128 nodes. 391 windows are balanced
    across 8 cores x 49 slots; each core aggregates its own windows entirely
    locally (no collectives).
  - Per (core, window, src-half) the edges are sorted by dst and cut into
    128-edge chunks (padded so chunk counts per slot are identical across
    cores -> one shared program).
  - Per chunk on device:
      scaling = relu([attr|1] @ [W_in;b_in])  (PE matmul, 2 chunks per matmul)
      msg     = scaling * x[src] (broadcast over hid=3)  (DVE, bf16)
      aggr   += onehot(dst_local)^T @ msg    (PE matmul into PSUM window)
    x[src] rows are fetched with dma_gather (SWDGE custom gather) from HBM.
    onehot matrices are host-precomputed fp8 and streamed in as weights.
  - Per window epilogue: transpose aggr, out = tanh(aggr @ W_out + b_out),
    DMA to a slot-indexed scratch; host unscrambles slots -> global rows.
"""

import math
import sys
from contextlib import ExitStack

import numpy as np

for _p in ("/opt/trn_rl_repo",):
    if _p not in sys.path:
        sys.path.insert(0, _p)

import ml_dtypes  # noqa: E402

import concourse.bacc as bacc  # noqa: E402
import concourse.bass as bass  # noqa: E402
import concourse.mybir as mybir  # noqa: E402
import concourse.tile as tile  # noqa: E402
from concourse.masks import make_identity  # noqa: E402

P = 128
F32 = mybir.dt.float32
F16 = mybir.dt.float16
FP8 = mybir.dt.float8e4
I16 = mybir.dt.int16

FP8_NP = mybir.dt.np(FP8)
F16_NP = np.float16


class Cfg:
    def __init__(self, n_nodes, c_in=64, hid=3, c_out=128, attr=6, n_cores=8,
                 gb=24, pb=24, eg=4, eg2=12, scratch=65536, ablate=(),
                 nq=4, aggr_bufs=2, epi_bufs=2, spsum_bufs=2, g128=False,
                 sbufs=4):
        self.n_nodes = n_nodes
        self.c_in = c_in            # 64
        self.hid = hid              # 3
        self.c_out = c_out          # 128
        self.attr = attr            # 6
        self.n_cores = n_cores
        self.half = (n_nodes + 1) // 2      # src split point for int16 gather
        self.n_win = (n_nodes + P - 1) // P
        self.n_slot = (self.n_win + n_cores - 1) // n_cores
        self.gb = gb                # chunks per gather / onehot batch
        self.pb = pb                # scaling pairs per attr batch
        self.eg = eg                # chunks per PSUM scaling group (eg/2 banks)
        self.eg2 = eg2              # chunks per elementwise supergroup
        self.sbufs = sbufs          # bufs for scal/xs/msg pools
        assert eg2 % eg == 0
        self.k192 = hid * c_in      # 192
        self.scratch = scratch
        self.ablate = set(ablate)
        self.nq = nq
        self.g128 = g128
        self.aggr_bufs = aggr_bufs
        self.epi_bufs = epi_bufs
        self.spsum_bufs = spsum_bufs
        assert (eg // 2) * spsum_bufs + aggr_bufs + epi_bufs <= 8
        # feature permutation: device feature j' = h*64+c  <->  ref j = 3c+h
        c = np.arange(c_in)
        self.perm = np.concatenate([hid * c + h for h in range(hid)])  # [192]


# --------------------------------------------------------------------------
# host-side scheduling / data prep
# --------------------------------------------------------------------------

class LayerSched:
    """Per-layer, cross-core-uniform chunk schedule + per-core data arrays."""
    pass


def _assign_windows(cfg, win_edge_counts):
    """Balance windows across cores. Returns win_of[core][slot] (-1 = pad)."""
    n_win, n_cores, n_slot = cfg.n_win, cfg.n_cores, cfg.n_slot
    order = np.argsort(-win_edge_counts, kind="stable")
    totals = np.zeros(n_cores, dtype=np.int64)
    counts = np.zeros(n_cores, dtype=np.int64)
    win_of = -np.ones((n_cores, n_slot), dtype=np.int64)
    for w in order:
        k = min((kk for kk in range(n_cores) if counts[kk] < n_slot),
                key=lambda kk: totals[kk])
        win_of[k, counts[k]] = w
        counts[k] += 1
        totals[k] += win_edge_counts[w]
    # within each core, sort slots by edge count desc so slot s across cores
    # holds comparably-sized windows (minimizes per-slot max padding)
    for k in range(n_cores):
        cnt = np.where(win_of[k] >= 0, win_edge_counts[np.maximum(win_of[k], 0)], -1)
        win_of[k] = win_of[k][np.argsort(-cnt, kind="stable")]
    return win_of


def prep_layer(cfg, edge_index, edge_attr, W_in, b_in, W_out, b_out):
    """Build the uniform schedule and per-core device arrays for one layer."""
    src = np.asarray(edge_index[0], dtype=np.int64).astype(np.int32)
    dst = np.asarray(edge_index[1], dtype=np.int64).astype(np.int32)
    attr = np.asarray(edge_attr, dtype=np.float32)
    E = src.shape[0]
    n_cores, n_slot = cfg.n_cores, cfg.n_slot

    win = dst >> 7
    if "oldassign" in cfg.ablate:
        win_counts = np.bincount(win, minlength=cfg.n_win)
        win_of = _assign_windows(cfg, win_counts)
        _skip_deal = True
    else:
        _skip_deal = False
    n_winp = cfg.n_cores * cfg.n_slot
    half_all = (src >= cfg.half)
    cl = np.bincount(win[~half_all], minlength=n_winp)
    chh = np.bincount(win[half_all], minlength=n_winp)
    clc, chc = (cl + P - 1) // P, (chh + P - 1) // P
    # deal windows sorted by chunk signature: slot s gets ranks 8s..8s+7,
    # one per core -> per-slot max over cores is tight
    if not _skip_deal:
        order = sorted(range(n_winp), key=lambda w: (int(clc[w]), int(chc[w])))
        win_of = -np.ones((cfg.n_cores, cfg.n_slot), dtype=np.int64)
        for i, w in enumerate(order):
            if w < cfg.n_win:
                win_of[i % cfg.n_cores, i // cfg.n_cores] = w

    # per (core, slot, half): edge index lists sorted by dst
    # edge order: sort once globally by (win, srchalf, dst)
    half = (src >= cfg.half).astype(np.int64)
    okey = (win.astype(np.int64) << 33) | (half << 32) | dst.astype(np.int64)
    eorder = np.argsort(okey, kind="stable")
    # boundaries per (win, half)
    wh = win.astype(np.int64) * 2 + half
    wh_sorted = wh[eorder]
    grp_start = {}
    uniq, starts = np.unique(wh_sorted, return_index=True)
    ends = np.append(starts[1:], E)
    for u, s0, e0 in zip(uniq, starts, ends):
        grp_start[int(u)] = (int(s0), int(e0))

    def group_edges(w, h):
        r = grp_start.get(int(w) * 2 + int(h))
        if r is None:
            return eorder[0:0]
        return eorder[r[0]:r[1]]

    # chunk counts per (core, slot, half)
    nch = np.zeros((n_cores, n_slot, 2), dtype=np.int64)
    for k in range(n_cores):
        for s in range(n_slot):
            w = win_of[k, s]
            if w < 0:
                continue
            for h in range(2):
                cnt = len(group_edges(w, h))
                nch[k, s, h] = (cnt + P - 1) // P
    CH = nch.max(axis=0)  # [n_slot, 2] uniform chunk counts
    for s in range(n_slot):
        if CH[s].sum() == 0:
            CH[s, 0] = 1  # every slot needs >=1 chunk to init its PSUM bank

    # global chunk schedule: per slot, lo-run then hi-run
    sched_chunks = []       # (slot, half)
    for s in range(n_slot):
        sched_chunks += [(s, 0)] * int(CH[s, 0]) + [(s, 1)] * int(CH[s, 1])
    # pad to multiple of eg2 (and so of 2) with hi-chunks on the last slot
    while len(sched_chunks) % cfg.eg2 != 0:
        sched_chunks.append((n_slot - 1, 1))
        CH[n_slot - 1, 1] += 1
    NCH = len(sched_chunks)

    # stream positions
    stream_pos = []         # per chunk: (half, pos in that stream)
    cnt_lo = cnt_hi = 0
    for (s, h) in sched_chunks:
        if h == 0:
            stream_pos.append((0, cnt_lo)); cnt_lo += 1
        else:
            stream_pos.append((1, cnt_hi)); cnt_hi += 1
    NLO, NHI = cnt_lo, cnt_hi
    NBLO = (NLO + cfg.gb - 1) // cfg.gb if NLO else 0
    NBHI = (NHI + cfg.gb - 1) // cfg.gb if NHI else 0
    NPAIR = NCH // 2
    NB14 = (NPAIR + cfg.pb - 1) // cfg.pb

    # slot boundaries: first/last chunk index per slot
    slot_first = {}
    slot_last = {}
    for ci, (s, h) in enumerate(sched_chunks):
        if s not in slot_first:
            slot_first[s] = ci
        slot_last[s] = ci

    sch = LayerSched()
    sch.cfg = cfg
    sch.win_of = win_of
    sch.CH = CH
    sch.chunks = sched_chunks
    sch.NCH = NCH
    sch.stream_pos = stream_pos
    sch.NLO, sch.NHI, sch.NBLO, sch.NBHI = NLO, NHI, NBLO, NBHI
    sch.NPAIR, sch.NB14 = NPAIR, NB14
    sch.slot_first, sch.slot_last = slot_first, slot_last

    # ---- weights (shared across cores)
    k192 = cfg.k192
    W7 = np.concatenate([np.asarray(W_in, np.float32),
                         np.asarray(b_in, np.float32)[None, :]], axis=0)  # [7,192]
    W7r = W7[:, cfg.perm]                                                 # [7,192]
    na = cfg.attr + 1
    W14 = np.zeros((2 * na, 2 * k192), dtype=np.float32)
    W14[:na, :k192] = W7r
    W14[na:, k192:] = W7r
    Woutr = np.asarray(W_out, np.float32)[cfg.perm, :]                    # [192,128]
    Wout1 = np.ascontiguousarray(Woutr[: k192 // 2])                      # [96,128]
    Wout2 = np.concatenate([Woutr[k192 // 2:],
                            np.asarray(b_out, np.float32)[None, :]], 0)   # [97,128]
    sch.W14, sch.Wout1, sch.Wout2 = W14, Wout1, Wout2

    # ---- per-core arrays
    sch.core = []
    iota128 = np.arange(P, dtype=np.int32)
    for k in range(n_cores):
        # flat per-chunk edge ids (-1 = pad)
        eids = -np.ones((NCH, P), dtype=np.int64)
        for s in range(n_slot):
            w = win_of[k, s]
            if w < 0:
                continue
            for h in range(2):
                g = group_edges(w, h)
                if len(g) == 0:
                    continue
                # chunk indices for this (s,h)
                base = slot_first[s] + (0 if h == 0 else int(CH[s, 0]))
                ncs = (len(g) + P - 1) // P
                for c in range(ncs):
                    eids[base + c, : len(g[c * P:(c + 1) * P])] = g[c * P:(c + 1) * P]
        valid = eids >= 0
        e_safe = np.maximum(eids, 0)

        srcs = np.where(valid, src[e_safe], 0)
        dsts = np.where(valid, dst[e_safe], 0)

        # gather indices per stream, wrapped [16, n/16] replicated to 128 parts
        def build_idx(nb, stream_h):
            arr = np.zeros((max(nb, 1), P, cfg.gb * 8), dtype=np.int16)
            if nb == 0:
                return arr
            flat = np.zeros(nb * cfg.gb * P, dtype=np.int16)
            pos = 0
            for ci in range(NCH):
                hh, pp = stream_pos[ci]
                if hh != stream_h:
                    continue
                v = srcs[ci].astype(np.int64) - (cfg.half if stream_h else 0)
                v = np.where(valid[ci], v, 0)
                flat[pp * P:(pp + 1) * P] = v.astype(np.int16)
                pos += 1
            wrapped = flat.reshape(nb, cfg.gb * 8, 16).transpose(0, 2, 1)  # [nb,16,gb*8]
            return np.ascontiguousarray(
                np.tile(wrapped, (1, 8, 1)).astype(np.int16))              # [nb,128,gb*8]

        idx_lo = build_idx(NBLO, 0)
        idx_hi = build_idx(NBHI, 1)

        # attr14: [NB14, 14, pb*128]
        attr14 = np.zeros((NB14, 2 * na, cfg.pb * P), dtype=np.float32)
        a7 = np.zeros((NCH, na, P), dtype=np.float32)
        a7[:, :cfg.attr, :] = np.where(valid[:, None, :],
                                       attr[e_safe].transpose(0, 2, 1), 0.0)
        a7[:, cfg.attr, :] = valid.astype(np.float32)
        for pr in range(NPAIR):
            b14, off = divmod(pr, cfg.pb)
            attr14[b14, :na, off * P:(off + 1) * P] = a7[2 * pr]
            attr14[b14, na:, off * P:(off + 1) * P] = a7[2 * pr + 1]

        # onehot fp8: [128, NCH*128] partition-major (batched by gb chunks)
        slot_ids = np.array([s for (s, h) in sched_chunks], dtype=np.int64)
        dloc = dsts - (win_of[k][slot_ids][:, None] * P)
        dloc = np.where(valid, dloc, -1)
        oh = (dloc[:, :, None] == iota128[None, None, :])                 # [NCH,128,128]
        oh8 = np.where(oh, np.uint8(0x38), np.uint8(0)).transpose(1, 0, 2)
        oh8 = oh8.reshape(P, NCH * P)
        nbo = (NCH + cfg.gb - 1) // cfg.gb
        if nbo * cfg.gb * P > NCH * P:  # pad to whole onehot batches
            pad = np.zeros((P, nbo * cfg.gb * P - NCH * P), dtype=np.uint8)
            oh8 = np.concatenate([oh8, pad], axis=1)
        oh8 = np.ascontiguousarray(oh8).view(FP8_NP)

        core = {"idx_lo": idx_lo, "idx_hi": idx_hi, "attr14": attr14, "oneh": oh8}
        sch.core.append(core)
    return sch


def _dma_gather_any(g, out_ap, in_ap, idxs_ap, num_idxs, num_idxs_reg,
                    elem_size, elem_step, single_packet=False, queue_num=0):
    """dma_gather without the elem_size%256 restriction (row stride must
    still be a 256B multiple; gathers the first elem_size elems per row)."""
    stride_bytes = elem_step * mybir.dt.size(in_ap.dtype)
    assert stride_bytes % 256 == 0 and stride_bytes // 256 < 256
    _in_ap = g.lower_ap_dma(in_ap, for_custom_bir_dma=True)
    _idxs_ap = g.lower_ap(idxs_ap)
    _out_ap = g.lower_ap(out_ap)
    return g.add_instruction(mybir.InstDMAGatherAnt(
        name=g.bass.get_next_instruction_name(),
        ins=[*_in_ap, _idxs_ap, g.lower_val_access(g.to_reg(num_idxs_reg))],
        outs=[_out_ap],
        transpose=False, num_idxs=num_idxs, elem_size=elem_size,
        stride_bytes_256=stride_bytes // 256, gen_mode=0,
        single_packet=single_packet, queue_num=queue_num,
        sbuf_tokens_per_rank=0, sbuf_free_dim_per_rank=0,
        sbuf_free_dim_pad_per_rank=0, sbuf_byte_offset=0))


# --------------------------------------------------------------------------
# device program
# --------------------------------------------------------------------------

def build_program(cfg, scheds, reps=1):
    """One shared SPMD program for all cores. scheds = [layer0, layer1]."""
    nc = bacc.Bacc("TRN2", target_bir_lowering=False, debug=False,
               dynamic_dma_scratch_size=cfg.scratch,
               num_swdge_queues=cfg.nq)
    n_layers = len(scheds)
    na = cfg.attr + 1
    k192, half_k = cfg.k192, cfg.k192 // 2

    # ---- DRAM tensors (identical shapes across cores)
    if cfg.g128:
        x_lo = nc.dram_tensor("x_lo", [cfg.half, 2 * cfg.c_in], F16,
                              kind="ExternalInput")
        x_hi = nc.dram_tensor("x_hi", [cfg.n_nodes - cfg.half, 2 * cfg.c_in],
                              F16, kind="ExternalInput")
    else:
        x_lo = nc.dram_tensor("x_lo", [cfg.half, cfg.c_in], F32,
                              kind="ExternalInput")
        x_hi = nc.dram_tensor("x_hi", [cfg.n_nodes - cfg.half, cfg.c_in], F32,
                              kind="ExternalInput")
    dins = []
    for li, sch in enumerate(scheds):
        d = {}
        d["idx_lo"] = nc.dram_tensor(f"idx_lo{li}", list(sch.core[0]["idx_lo"].shape),
                                     I16, kind="ExternalInput")
        d["idx_hi"] = nc.dram_tensor(f"idx_hi{li}", list(sch.core[0]["idx_hi"].shape),
                                     I16, kind="ExternalInput")
        d["attr14"] = nc.dram_tensor(f"attr14_{li}", list(sch.core[0]["attr14"].shape),
                                     F32, kind="ExternalInput")
        d["oneh"] = nc.dram_tensor(f"oneh{li}", list(sch.core[0]["oneh"].shape),
                                   FP8, kind="ExternalInput")
        d["W14"] = nc.dram_tensor(f"W14_{li}", list(sch.W14.shape), F32,
                                  kind="ExternalInput")
        d["Wout1"] = nc.dram_tensor(f"Wout1_{li}", list(sch.Wout1.shape), F32,
                                    kind="ExternalInput")
        d["Wout2"] = nc.dram_tensor(f"Wout2_{li}", list(sch.Wout2.shape), F32,
                                    kind="ExternalInput")
        dins.append(d)
    out_scr = nc.dram_tensor("out_scr", [n_layers, cfg.n_slot * P, cfg.c_out], F32,
                             kind="ExternalOutput")

    with tile.TileContext(nc) as tc, ExitStack() as ctx:
        const = ctx.enter_context(tc.tile_pool(name="const", bufs=1))
        idxp = (None if "nogather" in cfg.ablate else
                ctx.enter_context(tc.tile_pool(name="idx", bufs=2)))
        xjp = [ctx.enter_context(tc.tile_pool(name=f"xj{h}", bufs=2)) for h in range(2)]
        attrp = ctx.enter_context(tc.tile_pool(name="attr", bufs=2))
        onehp = ctx.enter_context(tc.tile_pool(name="oneh", bufs=2))
        spsum = ctx.enter_context(tc.tile_pool(name="spsum", bufs=cfg.spsum_bufs, space="PSUM"))
        aggrp = ctx.enter_context(tc.tile_pool(name="aggr", bufs=cfg.aggr_bufs, space="PSUM"))
        epip = ctx.enter_context(tc.tile_pool(name="epip", bufs=cfg.epi_bufs, space="PSUM"))
        scalp = ctx.enter_context(tc.tile_pool(name="scal", bufs=cfg.sbufs))
        xsp = ctx.enter_context(tc.tile_pool(name="xs", bufs=cfg.sbufs))
        msgp = ctx.enter_context(tc.tile_pool(name="msg", bufs=cfg.sbufs))
        aggsb = ctx.enter_context(tc.tile_pool(name="aggsb", bufs=2))
        atsb = ctx.enter_context(tc.tile_pool(name="atsb", bufs=2))
        outsb = ctx.enter_context(tc.tile_pool(name="outsb", bufs=2))

        ident = const.tile([P, P], F32)
        make_identity(nc, ident[:])

        wtiles = []
        for li, sch in enumerate(scheds):
            d = dins[li]
            w14 = const.tile([2 * na, 2 * k192], F32, tag=f"w14_{li}")
            nc.sync.dma_start(out=w14[:], in_=d["W14"][:])
            wo1 = const.tile([half_k, cfg.c_out], F32, tag=f"wo1_{li}")
            nc.sync.dma_start(out=wo1[:], in_=d["Wout1"][:])
            wo2 = const.tile([half_k + 1, cfg.c_out], F32, tag=f"wo2_{li}")
            nc.sync.dma_start(out=wo2[:], in_=d["Wout2"][:])
            wtiles.append((w14, wo1, wo2))

        def emit_layer(li, sch):
            d = dins[li]
            w14, wo1, wo2 = wtiles[li]

            xj_tile_of = [{}, {}]        # stream -> {batch index: tile}
            oneh_tile = None
            attr_tile = None
            attr_b14 = -1
            sp_tile = None               # scaling psum group tile
            scal_tile = xs_tile = msg_tile = None
            aggr_tile = None

            def ensure_gather(stream, bi):
                if bi in xj_tile_of[stream]:
                    return
                xt = xjp[stream].tile([P, cfg.gb, cfg.c_in],
                                      F16 if cfg.g128 else F32)
                if "nogather" in cfg.ablate:
                    nc.gpsimd.memset(xt[:, 0:1, :], 0.25)
                else:
                    it = idxp.tile([P, cfg.gb * 8], I16)
                    src_dram = d["idx_lo"] if stream == 0 else d["idx_hi"]
                    nc.sync.dma_start(out=it[:], in_=src_dram[bi])
                    table = x_lo if stream == 0 else x_hi
                    if cfg.g128:
                        _dma_gather_any(
                            nc.gpsimd, out_ap=xt[:],
                            in_ap=table[:, 0:cfg.c_in], idxs_ap=it[:],
                            num_idxs=cfg.gb * P, num_idxs_reg=cfg.gb * P,
                            elem_size=cfg.c_in, elem_step=2 * cfg.c_in,
                            single_packet=False,
                            queue_num=(stream * 7 + bi) % cfg.nq)
                    else:
                        nc.gpsimd.dma_gather(
                            out_ap=xt[:], in_ap=table[:], idxs_ap=it[:],
                            num_idxs=cfg.gb * P, num_idxs_reg=cfg.gb * P,
                            elem_size=cfg.c_in, single_packet=False,
                            queue_num=(stream * 7 + bi) % cfg.nq)
                xj_tile_of[stream][bi] = xt

            for ci in range(sch.NCH):
                s, h = sch.chunks[ci]

                # ---- supergroup head: gathers, scaling, relu, stage, mul
                if ci % cfg.eg2 == 0:
                    for cj in range(ci, ci + cfg.eg2):
                        st_j, pos_j = sch.stream_pos[cj]
                        ensure_gather(st_j, pos_j // cfg.gb)
                    scal_tile = scalp.tile([P, cfg.eg2, k192], F16)
                    for g0 in range(0, cfg.eg2, cfg.eg):
                        sp_tile = spsum.tile([P, cfg.eg // 2, 512], F32)
                        for pj in range(cfg.eg // 2):
                            pr = (ci + g0) // 2 + pj
                            b14, poff = divmod(pr, cfg.pb)
                            if b14 != attr_b14:
                                attr_tile = attrp.tile([2 * na, cfg.pb * P],
                                                       F32)
                                nc.sync.dma_start(out=attr_tile[:],
                                                  in_=d["attr14"][b14])
                                attr_b14 = b14
                            if "nosc" in cfg.ablate:
                                nc.vector.memset(sp_tile[:, pj, 0:8], 0.0)
                            else:
                                nc.tensor.matmul(
                                    sp_tile[:, pj, 0:2 * k192],
                                    attr_tile[:, poff * P:(poff + 1) * P],
                                    w14[:],
                                    start=True, stop=True)
                        if "noelem" in cfg.ablate:
                            nc.gpsimd.memset(
                                scal_tile[:, g0:g0 + 1, 0:8], 0.25)
                        else:
                            nc.scalar.activation(
                                scal_tile[:, g0:g0 + cfg.eg, :],
                                sp_tile[:, :, 0:2 * k192],
                                mybir.ActivationFunctionType.Relu)
                    # stage x_j for the eg2 chunks (contiguous runs per stream)
                    xs_tile = xsp.tile([P, cfg.eg2, cfg.c_in], F16)
                    j = 0
                    while j < cfg.eg2 and "noelem" not in cfg.ablate:
                        st_j, pos_j = sch.stream_pos[ci + j]
                        bi_j, off_j = divmod(pos_j, cfg.gb)
                        run = 1
                        while (j + run < cfg.eg2):
                            st_n, pos_n = sch.stream_pos[ci + j + run]
                            if st_n != st_j or pos_n != pos_j + run:
                                break
                            if divmod(pos_n, cfg.gb)[0] != bi_j:
                                break
                            run += 1
                        nc.vector.tensor_copy(
                            out=xs_tile[:, j:j + run, :],
                            in_=xj_tile_of[st_j][bi_j][:, off_j:off_j + run, :])
                        j += run
                    # msg = scal * xj  (3 ops, one per h block)
                    msg_tile = msgp.tile([P, cfg.eg2, k192], F16)
                    if "noelem" in cfg.ablate:
                        nc.gpsimd.memset(msg_tile[:, 0:1, 0:8], 0.25)
                    for hh in range(cfg.hid if "noelem" not in cfg.ablate else 0):
                        o0 = hh * cfg.c_in
                        nc.vector.tensor_tensor(
                            out=msg_tile[:, :, o0:o0 + cfg.c_in],
                            in0=scal_tile[:, :, o0:o0 + cfg.c_in],
                            in1=xs_tile[:],
                            op=mybir.AluOpType.mult)

                # ---- onehot batch load
                obi, ooff = divmod(ci, cfg.gb)
                if ooff == 0:
                    oneh_tile = onehp.tile([P, cfg.gb * P], FP8)
                    nc.sync.dma_start(
                        out=oneh_tile[:],
                        in_=d["oneh"][:, obi * cfg.gb * P:(obi + 1) * cfg.gb * P])

                eoff = ci % cfg.eg2
                # ---- scatter matmul into the slot's aggr window
                if "noscatter" in cfg.ablate:
                    continue
                if ci == sch.slot_first[s]:
                    aggr_tile = aggrp.tile([P, 512], F32)
                nc.tensor.matmul(
                    aggr_tile[:, 0:k192],
                    oneh_tile[:, ooff * P:(ooff + 1) * P],
                    msg_tile[:, eoff, :],
                    start=(ci == sch.slot_first[s]),
                    stop=(ci == sch.slot_last[s]))

                # ---- epilogue at slot end
                if ci == sch.slot_last[s]:
                    asb = aggsb.tile([P, k192], F32)
                    nc.scalar.activation(asb[:], aggr_tile[:, 0:k192],
                                         mybir.ActivationFunctionType.Copy)
                    ep = epip.tile([P, 512], F32)
                    at1 = atsb.tile([half_k, P], F32, tag="at1")
                    at2 = atsb.tile([half_k + 1, P], F32, tag="at2")
                    nc.tensor.transpose(ep[0:half_k, 0:P], asb[:, 0:half_k],
                                        ident[:])
                    nc.vector.tensor_copy(out=at1[:], in_=ep[0:half_k, 0:P])
                    nc.tensor.transpose(ep[0:half_k, P:2 * P],
                                        asb[:, half_k:k192], ident[:])
                    nc.vector.tensor_copy(out=at2[0:half_k, :],
                                          in_=ep[0:half_k, P:2 * P])
                    nc.vector.memset(at2[half_k:half_k + 1, :], 1.0)
                    nc.tensor.matmul(ep[:, 2 * P:3 * P], at1[:], wo1[:],
                                     start=True, stop=False)
                    nc.tensor.matmul(ep[:, 2 * P:3 * P], at2[:], wo2[:],
                                     start=False, stop=True)
                    osb = outsb.tile([P, cfg.c_out], F32)
                    nc.scalar.activation(osb[:], ep[:, 2 * P:3 * P],
                                         mybir.ActivationFunctionType.Tanh)
                    nc.sync.dma_start(
                        out=out_scr[li, s * P:(s + 1) * P, :], in_=osb[:])

        if reps > 1:
            with tc.For_i(0, reps, 1):
                for li, sch in enumerate(scheds):
                    emit_layer(li, sch)
        else:
            for li, sch in enumerate(scheds):
                emit_layer(li, sch)
    nc.finalize()
    return nc


# --------------------------------------------------------------------------
# entry point
# --------------------------------------------------------------------------

def make_in_maps(cfg, x, scheds):
    x = np.asarray(x, np.float32)
    if cfg.g128:
        xp = np.zeros((cfg.n_nodes, 2 * cfg.c_in), dtype=np.float16)
        xp[:, :cfg.c_in] = x.astype(np.float16)
        tlo = np.ascontiguousarray(xp[:cfg.half])
        thi = np.ascontiguousarray(xp[cfg.half:])
    else:
        tlo = np.ascontiguousarray(x[:cfg.half])
        thi = np.ascontiguousarray(x[cfg.half:])
    in_maps = []
    for k in range(cfg.n_cores):
        m = {"x_lo": tlo, "x_hi": thi}
        for li, sch in enumerate(scheds):
            c = sch.core[k]
            m[f"idx_lo{li}"] = c["idx_lo"]
            m[f"idx_hi{li}"] = c["idx_hi"]
            m[f"attr14_{li}"] = c["attr14"]
            m[f"oneh{li}"] = c["oneh"]
            m[f"W14_{li}"] = sch.W14
            m[f"Wout1_{li}"] = sch.Wout1
            m[f"Wout2_{li}"] = sch.Wout2
        in_maps.append(m)
    return in_maps


def _run(cfg, x, layers, reps=1):
    """layers: list of (edge_index, edge_attr, W_in, b_in, W_out, b_out)."""
    scheds = [prep_layer(cfg, *lay) for lay in layers]
    nc = build_program(cfg, scheds, reps=reps)
    in_maps = make_in_maps(cfg, x, scheds)

    from concourse.bass_utils import run_bass_kernel_spmd
    res = run_bass_kernel_spmd(nc, in_maps, list(range(cfg.n_cores)),
                               trace=globals().get("TRACE", False))

    n_layers = len(layers)
    out = np.zeros((cfg.n_nodes, n_layers * cfg.c_out), dtype=np.float32)
    for k in range(cfg.n_cores):
        scr = res.results[k]["out_scr"]
        for li, sch in enumerate(scheds):
            for s in range(cfg.n_slot):
                w = sch.win_of[k, s]
                if w < 0:
                    continue
                r0 = int(w) * P
                r1 = min(r0 + P, cfg.n_nodes)
                out[r0:r1, li * cfg.c_out:(li + 1) * cfg.c_out] = \
                    scr[li, s * P:s * P + (r1 - r0), :]
    return out, res


def kernel(x, edge_index0, edge_attr0, edge_index1, edge_attr1,
           W_in0, b_in0, W_out0, b_out0, W_in1, b_in1, W_out1, b_out1):
    x = np.asarray(x)
    cfg = Cfg(n_nodes=x.shape[0], c_in=x.shape[1])
    layers = [
        (np.asarray(edge_index0), np.asarray(edge_attr0),
         np.asarray(W_in0), np.asarray(b_in0),
         np.asarray(W_out0), np.asarray(b_out0)),
        (np.asarray(edge_index1), np.asarray(edge_attr1),
         np.asarray(W_in1), np.asarray(b_in1),
         np.asarray(W_out1), np.asarray(b_out1)),
    ]
    out, _ = _run(cfg, x, layers)
    return out



# revision 9
# speedup vs baseline: 1.1779x; 1.1779x over previous
"""Trainium2 Bass kernel for nn_CFDFVGCN (two SpatialGraphConv layers, concat).

Strategy (8 NeuronCores, SPMD single program):
  - Shard by DESTINATION node windows of 128 nodes. 391 windows are balanced
    across 8 cores x 49 slots; each core aggregates its own windows entirely
    locally (no collectives).
  - Per (core, window, src-half) the edges are sorted by dst and cut into
    128-edge chunks (padded so chunk counts per slot are identical across
    cores -> one shared program).
  - Per chunk on device:
      scaling = relu([attr|1] @ [W_in;b_in])  (PE matmul, 2 chunks per matmul)
      msg     = scaling * x[src] (broadcast over hid=3)  (DVE, bf16)
      aggr   += onehot(dst_local)^T @ msg    (PE matmul into PSUM window)
    x[src] rows are fetched with dma_gather (SWDGE custom gather) from HBM.
    onehot matrices are host-precomputed fp8 and streamed in as weights.
  - Per window epilogue: transpose aggr, out = tanh(aggr @ W_out + b_out),
    DMA to a slot-indexed scratch; host unscrambles slots -> global rows.
"""

import math
import sys
from contextlib import ExitStack

import numpy as np

for _p in ("/opt/trn_rl_repo",):
    if _p not in sys.path:
        sys.path.insert(0, _p)

import ml_dtypes  # noqa: E402

import concourse.bacc as bacc  # noqa: E402
import concourse.bass as bass  # noqa: E402
import concourse.mybir as mybir  # noqa: E402
import concourse.tile as tile  # noqa: E402
from concourse.masks import make_identity  # noqa: E402

P = 128
F32 = mybir.dt.float32
F16 = mybir.dt.float16
FP8 = mybir.dt.float8e4
I16 = mybir.dt.int16

FP8_NP = mybir.dt.np(FP8)
F16_NP = np.float16


class Cfg:
    def __init__(self, n_nodes, c_in=64, hid=3, c_out=128, attr=6, n_cores=8,
                 gb=24, pb=24, eg=4, eg2=12, scratch=65536, ablate=(),
                 nq=4, aggr_bufs=2, epi_bufs=2, spsum_bufs=2, g128=True,
                 sbufs=4):
        self.n_nodes = n_nodes
        self.c_in = c_in            # 64
        self.hid = hid              # 3
        self.c_out = c_out          # 128
        self.attr = attr            # 6
        self.n_cores = n_cores
        self.half = (n_nodes + 1) // 2      # src split point for int16 gather
        self.n_win = (n_nodes + P - 1) // P
        self.n_slot = (self.n_win + n_cores - 1) // n_cores
        self.gb = gb                # chunks per gather / onehot batch
        self.pb = pb                # scaling pairs per attr batch
        self.eg = eg                # chunks per PSUM scaling group (eg/2 banks)
        self.eg2 = eg2              # chunks per elementwise supergroup
        self.sbufs = sbufs          # bufs for scal/xs/msg pools
        assert eg2 % eg == 0
        self.k192 = hid * c_in      # 192
        self.scratch = scratch
        self.ablate = set(ablate)
        self.nq = nq
        self.g128 = g128
        self.aggr_bufs = aggr_bufs
        self.epi_bufs = epi_bufs
        self.spsum_bufs = spsum_bufs
        assert (eg // 2) * spsum_bufs + aggr_bufs + epi_bufs <= 8
        # feature permutation: device feature j' = h*64+c  <->  ref j = 3c+h
        c = np.arange(c_in)
        self.perm = np.concatenate([hid * c + h for h in range(hid)])  # [192]


# --------------------------------------------------------------------------
# host-side scheduling / data prep
# --------------------------------------------------------------------------

class LayerSched:
    """Per-layer, cross-core-uniform chunk schedule + per-core data arrays."""
    pass


def _assign_windows(cfg, win_edge_counts):
    """Balance windows across cores. Returns win_of[core][slot] (-1 = pad)."""
    n_win, n_cores, n_slot = cfg.n_win, cfg.n_cores, cfg.n_slot
    order = np.argsort(-win_edge_counts, kind="stable")
    totals = np.zeros(n_cores, dtype=np.int64)
    counts = np.zeros(n_cores, dtype=np.int64)
    win_of = -np.ones((n_cores, n_slot), dtype=np.int64)
    for w in order:
        k = min((kk for kk in range(n_cores) if counts[kk] < n_slot),
                key=lambda kk: totals[kk])
        win_of[k, counts[k]] = w
        counts[k] += 1
        totals[k] += win_edge_counts[w]
    # within each core, sort slots by edge count desc so slot s across cores
    # holds comparably-sized windows (minimizes per-slot max padding)
    for k in range(n_cores):
        cnt = np.where(win_of[k] >= 0, win_edge_counts[np.maximum(win_of[k], 0)], -1)
        win_of[k] = win_of[k][np.argsort(-cnt, kind="stable")]
    return win_of


def prep_layer(cfg, edge_index, edge_attr, W_in, b_in, W_out, b_out):
    """Build the uniform schedule and per-core device arrays for one layer."""
    src = np.asarray(edge_index[0], dtype=np.int64).astype(np.int32)
    dst = np.asarray(edge_index[1], dtype=np.int64).astype(np.int32)
    attr = np.asarray(edge_attr, dtype=np.float32)
    E = src.shape[0]
    n_cores, n_slot = cfg.n_cores, cfg.n_slot

    win = dst >> 7
    if "oldassign" in cfg.ablate:
        win_counts = np.bincount(win, minlength=cfg.n_win)
        win_of = _assign_windows(cfg, win_counts)
        _skip_deal = True
    else:
        _skip_deal = False
    n_winp = cfg.n_cores * cfg.n_slot
    half_all = (src >= cfg.half)
    cl = np.bincount(win[~half_all], minlength=n_winp)
    chh = np.bincount(win[half_all], minlength=n_winp)
    clc, chc = (cl + P - 1) // P, (chh + P - 1) // P
    # deal windows sorted by chunk signature: slot s gets ranks 8s..8s+7,
    # one per core -> per-slot max over cores is tight
    if not _skip_deal:
        order = sorted(range(n_winp), key=lambda w: (int(clc[w]), int(chc[w])))
        win_of = -np.ones((cfg.n_cores, cfg.n_slot), dtype=np.int64)
        for i, w in enumerate(order):
            if w < cfg.n_win:
                win_of[i % cfg.n_cores, i // cfg.n_cores] = w

    # per (core, slot, half): edge index lists sorted by dst
    # edge order: sort once globally by (win, srchalf, dst)
    half = (src >= cfg.half).astype(np.int64)
    okey = (win.astype(np.int64) << 33) | (half << 32) | dst.astype(np.int64)
    eorder = np.argsort(okey, kind="stable")
    # boundaries per (win, half)
    wh = win.astype(np.int64) * 2 + half
    wh_sorted = wh[eorder]
    grp_start = {}
    uniq, starts = np.unique(wh_sorted, return_index=True)
    ends = np.append(starts[1:], E)
    for u, s0, e0 in zip(uniq, starts, ends):
        grp_start[int(u)] = (int(s0), int(e0))

    def group_edges(w, h):
        r = grp_start.get(int(w) * 2 + int(h))
        if r is None:
            return eorder[0:0]
        return eorder[r[0]:r[1]]

    # chunk counts per (core, slot, half)
    nch = np.zeros((n_cores, n_slot, 2), dtype=np.int64)
    for k in range(n_cores):
        for s in range(n_slot):
            w = win_of[k, s]
            if w < 0:
                continue
            for h in range(2):
                cnt = len(group_edges(w, h))
                nch[k, s, h] = (cnt + P - 1) // P
    CH = nch.max(axis=0)  # [n_slot, 2] uniform chunk counts
    for s in range(n_slot):
        if CH[s].sum() == 0:
            CH[s, 0] = 1  # every slot needs >=1 chunk to init its PSUM bank

    # global chunk schedule: per slot, lo-run then hi-run
    sched_chunks = []       # (slot, half)
    for s in range(n_slot):
        sched_chunks += [(s, 0)] * int(CH[s, 0]) + [(s, 1)] * int(CH[s, 1])
    # pad to multiple of eg2 (and so of 2) with hi-chunks on the last slot
    while len(sched_chunks) % cfg.eg2 != 0:
        sched_chunks.append((n_slot - 1, 1))
        CH[n_slot - 1, 1] += 1
    NCH = len(sched_chunks)

    # stream positions
    stream_pos = []         # per chunk: (half, pos in that stream)
    cnt_lo = cnt_hi = 0
    for (s, h) in sched_chunks:
        if h == 0:
            stream_pos.append((0, cnt_lo)); cnt_lo += 1
        else:
            stream_pos.append((1, cnt_hi)); cnt_hi += 1
    NLO, NHI = cnt_lo, cnt_hi
    NBLO = (NLO + cfg.gb - 1) // cfg.gb if NLO else 0
    NBHI = (NHI + cfg.gb - 1) // cfg.gb if NHI else 0
    NPAIR = NCH // 2
    NB14 = (NPAIR + cfg.pb - 1) // cfg.pb

    # slot boundaries: first/last chunk index per slot
    slot_first = {}
    slot_last = {}
    for ci, (s, h) in enumerate(sched_chunks):
        if s not in slot_first:
            slot_first[s] = ci
        slot_last[s] = ci

    sch = LayerSched()
    sch.cfg = cfg
    sch.win_of = win_of
    sch.CH = CH
    sch.chunks = sched_chunks
    sch.NCH = NCH
    sch.stream_pos = stream_pos
    sch.NLO, sch.NHI, sch.NBLO, sch.NBHI = NLO, NHI, NBLO, NBHI
    sch.NPAIR, sch.NB14 = NPAIR, NB14
    sch.slot_first, sch.slot_last = slot_first, slot_last

    # ---- weights (shared across cores)
    k192 = cfg.k192
    W7 = np.concatenate([np.asarray(W_in, np.float32),
                         np.asarray(b_in, np.float32)[None, :]], axis=0)  # [7,192]
    W7r = W7[:, cfg.perm]                                                 # [7,192]
    na = cfg.attr + 1
    W14 = np.zeros((2 * na, 2 * k192), dtype=np.float32)
    W14[:na, :k192] = W7r
    W14[na:, k192:] = W7r
    Woutr = np.asarray(W_out, np.float32)[cfg.perm, :]                    # [192,128]
    Wout1 = np.ascontiguousarray(Woutr[: k192 // 2])                      # [96,128]
    Wout2 = np.concatenate([Woutr[k192 // 2:],
                            np.asarray(b_out, np.float32)[None, :]], 0)   # [97,128]
    sch.W14 = W14.astype(np.float16)
    sch.Wout1 = Wout1.astype(np.float16)
    sch.Wout2 = Wout2.astype(np.float16)

    # ---- per-core arrays
    sch.core = []
    iota128 = np.arange(P, dtype=np.int32)
    for k in range(n_cores):
        # flat per-chunk edge ids (-1 = pad)
        eids = -np.ones((NCH, P), dtype=np.int64)
        for s in range(n_slot):
            w = win_of[k, s]
            if w < 0:
                continue
            for h in range(2):
                g = group_edges(w, h)
                if len(g) == 0:
                    continue
                # chunk indices for this (s,h)
                base = slot_first[s] + (0 if h == 0 else int(CH[s, 0]))
                ncs = (len(g) + P - 1) // P
                for c in range(ncs):
                    eids[base + c, : len(g[c * P:(c + 1) * P])] = g[c * P:(c + 1) * P]
        valid = eids >= 0
        e_safe = np.maximum(eids, 0)

        srcs = np.where(valid, src[e_safe], 0)
        dsts = np.where(valid, dst[e_safe], 0)

        # gather indices per stream, wrapped [16, n/16] replicated to 128 parts
        def build_idx(nb, stream_h):
            arr = np.zeros((max(nb, 1), P, cfg.gb * 8), dtype=np.int16)
            if nb == 0:
                return arr
            flat = np.zeros(nb * cfg.gb * P, dtype=np.int16)
            pos = 0
            for ci in range(NCH):
                hh, pp = stream_pos[ci]
                if hh != stream_h:
                    continue
                v = srcs[ci].astype(np.int64) - (cfg.half if stream_h else 0)
                v = np.where(valid[ci], v, 0)
                flat[pp * P:(pp + 1) * P] = v.astype(np.int16)
                pos += 1
            wrapped = flat.reshape(nb, cfg.gb * 8, 16).transpose(0, 2, 1)  # [nb,16,gb*8]
            return np.ascontiguousarray(
                np.tile(wrapped, (1, 8, 1)).astype(np.int16))              # [nb,128,gb*8]

        idx_lo = build_idx(NBLO, 0)
        idx_hi = build_idx(NBHI, 1)

        # attr14: [NB14, 14, pb*128]
        attr14 = np.zeros((NB14, 2 * na, cfg.pb * P), dtype=np.float16)
        a7 = np.zeros((NCH, na, P), dtype=np.float32)
        a7[:, :cfg.attr, :] = np.where(valid[:, None, :],
                                       attr[e_safe].transpose(0, 2, 1), 0.0)
        a7[:, cfg.attr, :] = valid.astype(np.float32)
        for pr in range(NPAIR):
            b14, off = divmod(pr, cfg.pb)
            attr14[b14, :na, off * P:(off + 1) * P] = a7[2 * pr]
            attr14[b14, na:, off * P:(off + 1) * P] = a7[2 * pr + 1]

        # onehot fp8: [128, NCH*128] partition-major (batched by gb chunks)
        slot_ids = np.array([s for (s, h) in sched_chunks], dtype=np.int64)
        dloc = dsts - (win_of[k][slot_ids][:, None] * P)
        dloc = np.where(valid, dloc, -1)
        oh = (dloc[:, :, None] == iota128[None, None, :])                 # [NCH,128,128]
        oh8 = np.where(oh, np.uint8(0x38), np.uint8(0)).transpose(1, 0, 2)
        oh8 = oh8.reshape(P, NCH * P)
        nbo = (NCH + cfg.gb - 1) // cfg.gb
        if nbo * cfg.gb * P > NCH * P:  # pad to whole onehot batches
            pad = np.zeros((P, nbo * cfg.gb * P - NCH * P), dtype=np.uint8)
            oh8 = np.concatenate([oh8, pad], axis=1)
        oh8 = np.ascontiguousarray(oh8).view(FP8_NP)

        core = {"idx_lo": idx_lo, "idx_hi": idx_hi, "attr14": attr14, "oneh": oh8}
        sch.core.append(core)
    return sch


def _dma_gather_any(g, out_ap, in_ap, idxs_ap, num_idxs, num_idxs_reg,
                    elem_size, elem_step, single_packet=False, queue_num=0):
    """dma_gather without the elem_size%256 restriction (row stride must
    still be a 256B multiple; gathers the first elem_size elems per row)."""
    stride_bytes = elem_step * mybir.dt.size(in_ap.dtype)
    assert stride_bytes % 256 == 0 and stride_bytes // 256 < 256
    _in_ap = g.lower_ap_dma(in_ap, for_custom_bir_dma=True)
    _idxs_ap = g.lower_ap(idxs_ap)
    _out_ap = g.lower_ap(out_ap)
    return g.add_instruction(mybir.InstDMAGatherAnt(
        name=g.bass.get_next_instruction_name(),
        ins=[*_in_ap, _idxs_ap, g.lower_val_access(g.to_reg(num_idxs_reg))],
        outs=[_out_ap],
        transpose=False, num_idxs=num_idxs, elem_size=elem_size,
        stride_bytes_256=stride_bytes // 256, gen_mode=0,
        single_packet=single_packet, queue_num=queue_num,
        sbuf_tokens_per_rank=0, sbuf_free_dim_per_rank=0,
        sbuf_free_dim_pad_per_rank=0, sbuf_byte_offset=0))


# --------------------------------------------------------------------------
# device program
# --------------------------------------------------------------------------

def build_program(cfg, scheds, reps=1):
    """One shared SPMD program for all cores. scheds = [layer0, layer1]."""
    nc = bacc.Bacc("TRN2", target_bir_lowering=False, debug=False,
               dynamic_dma_scratch_size=cfg.scratch,
               num_swdge_queues=cfg.nq)
    n_layers = len(scheds)
    na = cfg.attr + 1
    k192, half_k = cfg.k192, cfg.k192 // 2

    # ---- DRAM tensors (identical shapes across cores)
    if cfg.g128:
        x_lo = nc.dram_tensor("x_lo", [cfg.half, 2 * cfg.c_in], F16,
                              kind="ExternalInput")
        x_hi = nc.dram_tensor("x_hi", [cfg.n_nodes - cfg.half, 2 * cfg.c_in],
                              F16, kind="ExternalInput")
    else:
        x_lo = nc.dram_tensor("x_lo", [cfg.half, cfg.c_in], F32,
                              kind="ExternalInput")
        x_hi = nc.dram_tensor("x_hi", [cfg.n_nodes - cfg.half, cfg.c_in], F32,
                              kind="ExternalInput")
    dins = []
    for li, sch in enumerate(scheds):
        d = {}
        d["idx_lo"] = nc.dram_tensor(f"idx_lo{li}", list(sch.core[0]["idx_lo"].shape),
                                     I16, kind="ExternalInput")
        d["idx_hi"] = nc.dram_tensor(f"idx_hi{li}", list(sch.core[0]["idx_hi"].shape),
                                     I16, kind="ExternalInput")
        d["attr14"] = nc.dram_tensor(f"attr14_{li}", list(sch.core[0]["attr14"].shape),
                                     F16, kind="ExternalInput")
        d["oneh"] = nc.dram_tensor(f"oneh{li}", list(sch.core[0]["oneh"].shape),
                                   FP8, kind="ExternalInput")
        d["W14"] = nc.dram_tensor(f"W14_{li}", list(sch.W14.shape), F16,
                                  kind="ExternalInput")
        d["Wout1"] = nc.dram_tensor(f"Wout1_{li}", list(sch.Wout1.shape), F16,
                                    kind="ExternalInput")
        d["Wout2"] = nc.dram_tensor(f"Wout2_{li}", list(sch.Wout2.shape), F16,
                                    kind="ExternalInput")
        dins.append(d)
    out_scr = nc.dram_tensor("out_scr", [n_layers, cfg.n_slot * P, cfg.c_out], F32,
                             kind="ExternalOutput")

    with tile.TileContext(nc) as tc, ExitStack() as ctx:
        const = ctx.enter_context(tc.tile_pool(name="const", bufs=1))
        idxp = (None if "nogather" in cfg.ablate else
                ctx.enter_context(tc.tile_pool(name="idx", bufs=2)))
        xjp = [ctx.enter_context(tc.tile_pool(name=f"xj{h}", bufs=2)) for h in range(2)]
        attrp = ctx.enter_context(tc.tile_pool(name="attr", bufs=2))
        onehp = ctx.enter_context(tc.tile_pool(name="oneh", bufs=2))
        spsum = ctx.enter_context(tc.tile_pool(name="spsum", bufs=cfg.spsum_bufs, space="PSUM"))
        aggrp = ctx.enter_context(tc.tile_pool(name="aggr", bufs=cfg.aggr_bufs, space="PSUM"))
        epip = ctx.enter_context(tc.tile_pool(name="epip", bufs=cfg.epi_bufs, space="PSUM"))
        scalp = ctx.enter_context(tc.tile_pool(name="scal", bufs=cfg.sbufs))
        xsp = ctx.enter_context(tc.tile_pool(name="xs", bufs=cfg.sbufs))
        msgp = ctx.enter_context(tc.tile_pool(name="msg", bufs=cfg.sbufs))
        aggsb = ctx.enter_context(tc.tile_pool(name="aggsb", bufs=2))
        atsb = ctx.enter_context(tc.tile_pool(name="atsb", bufs=2))
        outsb = ctx.enter_context(tc.tile_pool(name="outsb", bufs=2))

        ident = const.tile([P, P], F32)
        make_identity(nc, ident[:])

        wtiles = []
        for li, sch in enumerate(scheds):
            d = dins[li]
            w14 = const.tile([2 * na, 2 * k192], F16, tag=f"w14_{li}")
            nc.sync.dma_start(out=w14[:], in_=d["W14"][:])
            wo1 = const.tile([half_k, cfg.c_out], F16, tag=f"wo1_{li}")
            nc.sync.dma_start(out=wo1[:], in_=d["Wout1"][:])
            wo2 = const.tile([half_k + 1, cfg.c_out], F16, tag=f"wo2_{li}")
            nc.sync.dma_start(out=wo2[:], in_=d["Wout2"][:])
            wtiles.append((w14, wo1, wo2))

        def emit_layer(li, sch):
            d = dins[li]
            w14, wo1, wo2 = wtiles[li]

            xj_tile_of = [{}, {}]        # stream -> {batch index: tile}
            oneh_tile = None
            attr_tile = None
            attr_b14 = -1
            sp_tile = None               # scaling psum group tile
            scal_tile = xs_tile = msg_tile = None
            aggr_tile = None

            def ensure_gather(stream, bi):
                if bi in xj_tile_of[stream]:
                    return
                xt = xjp[stream].tile([P, cfg.gb, cfg.c_in],
                                      F16 if cfg.g128 else F32)
                if "nogather" in cfg.ablate:
                    nc.gpsimd.memset(xt[:, 0:1, :], 0.25)
                else:
                    it = idxp.tile([P, cfg.gb * 8], I16)
                    src_dram = d["idx_lo"] if stream == 0 else d["idx_hi"]
                    nc.sync.dma_start(out=it[:], in_=src_dram[bi])
                    table = x_lo if stream == 0 else x_hi
                    if cfg.g128:
                        _dma_gather_any(
                            nc.gpsimd, out_ap=xt[:],
                            in_ap=table[:, 0:cfg.c_in], idxs_ap=it[:],
                            num_idxs=cfg.gb * P, num_idxs_reg=cfg.gb * P,
                            elem_size=cfg.c_in, elem_step=2 * cfg.c_in,
                            single_packet=False,
                            queue_num=(stream * 7 + bi) % cfg.nq)
                    else:
                        nc.gpsimd.dma_gather(
                            out_ap=xt[:], in_ap=table[:], idxs_ap=it[:],
                            num_idxs=cfg.gb * P, num_idxs_reg=cfg.gb * P,
                            elem_size=cfg.c_in, single_packet=False,
                            queue_num=(stream * 7 + bi) % cfg.nq)
                xj_tile_of[stream][bi] = xt

            for ci in range(sch.NCH):
                s, h = sch.chunks[ci]

                # ---- supergroup head: gathers, scaling, relu, stage, mul
                if ci % cfg.eg2 == 0:
                    for cj in range(ci, ci + cfg.eg2):
                        st_j, pos_j = sch.stream_pos[cj]
                        ensure_gather(st_j, pos_j // cfg.gb)
                    scal_tile = scalp.tile([P, cfg.eg2, k192], F16)
                    for g0 in range(0, cfg.eg2, cfg.eg):
                        sp_tile = spsum.tile([P, cfg.eg // 2, 512], F32)
                        for pj in range(cfg.eg // 2):
                            pr = (ci + g0) // 2 + pj
                            b14, poff = divmod(pr, cfg.pb)
                            if b14 != attr_b14:
                                attr_tile = attrp.tile([2 * na, cfg.pb * P],
                                                       F16)
                                nc.sync.dma_start(out=attr_tile[:],
                                                  in_=d["attr14"][b14])
                                attr_b14 = b14
                            if "nosc" in cfg.ablate:
                                nc.vector.memset(sp_tile[:, pj, 0:8], 0.0)
                            else:
                                nc.tensor.matmul(
                                    sp_tile[:, pj, 0:2 * k192],
                                    attr_tile[:, poff * P:(poff + 1) * P],
                                    w14[:],
                                    start=True, stop=True)
                        if "noelem" in cfg.ablate:
                            nc.gpsimd.memset(
                                scal_tile[:, g0:g0 + 1, 0:8], 0.25)
                        else:
                            nc.scalar.activation(
                                scal_tile[:, g0:g0 + cfg.eg, :],
                                sp_tile[:, :, 0:2 * k192],
                                mybir.ActivationFunctionType.Relu)
                    # stage x_j for the eg2 chunks (contiguous runs per stream)
                    xs_tile = xsp.tile([P, cfg.eg2, cfg.c_in], F16)
                    j = 0
                    while j < cfg.eg2 and "noelem" not in cfg.ablate:
                        st_j, pos_j = sch.stream_pos[ci + j]
                        bi_j, off_j = divmod(pos_j, cfg.gb)
                        run = 1
                        while (j + run < cfg.eg2):
                            st_n, pos_n = sch.stream_pos[ci + j + run]
                            if st_n != st_j or pos_n != pos_j + run:
                                break
                            if divmod(pos_n, cfg.gb)[0] != bi_j:
                                break
                            run += 1
                        nc.vector.tensor_copy(
                            out=xs_tile[:, j:j + run, :],
                            in_=xj_tile_of[st_j][bi_j][:, off_j:off_j + run, :])
                        j += run
                    # msg = scal * xj  (3 ops, one per h block)
                    msg_tile = msgp.tile([P, cfg.eg2, k192], F16)
                    if "noelem" in cfg.ablate:
                        nc.gpsimd.memset(msg_tile[:, 0:1, 0:8], 0.25)
                    for hh in range(cfg.hid if "noelem" not in cfg.ablate else 0):
                        o0 = hh * cfg.c_in
                        nc.vector.tensor_tensor(
                            out=msg_tile[:, :, o0:o0 + cfg.c_in],
                            in0=scal_tile[:, :, o0:o0 + cfg.c_in],
                            in1=xs_tile[:],
                            op=mybir.AluOpType.mult)

                # ---- onehot batch load
                obi, ooff = divmod(ci, cfg.gb)
                if ooff == 0:
                    oneh_tile = onehp.tile([P, cfg.gb * P], FP8)
                    nc.sync.dma_start(
                        out=oneh_tile[:],
                        in_=d["oneh"][:, obi * cfg.gb * P:(obi + 1) * cfg.gb * P])

                eoff = ci % cfg.eg2
                # ---- scatter matmul into the slot's aggr window
                if "noscatter" in cfg.ablate:
                    continue
                if ci == sch.slot_first[s]:
                    aggr_tile = aggrp.tile([P, 512], F32)
                nc.tensor.matmul(
                    aggr_tile[:, 0:k192],
                    oneh_tile[:, ooff * P:(ooff + 1) * P],
                    msg_tile[:, eoff, :],
                    start=(ci == sch.slot_first[s]),
                    stop=(ci == sch.slot_last[s]))

                # ---- epilogue at slot end
                if ci == sch.slot_last[s]:
                    asb = aggsb.tile([P, k192], F32)
                    nc.scalar.activation(asb[:], aggr_tile[:, 0:k192],
                                         mybir.ActivationFunctionType.Copy)
                    ep = epip.tile([P, 512], F32)
                    at1 = atsb.tile([half_k, P], F16, tag="at1")
                    at2 = atsb.tile([half_k + 1, P], F16, tag="at2")
                    nc.tensor.transpose(ep[0:half_k, 0:P], asb[:, 0:half_k],
                                        ident[:])
                    nc.vector.tensor_copy(out=at1[:], in_=ep[0:half_k, 0:P])
                    nc.tensor.transpose(ep[0:half_k, P:2 * P],
                                        asb[:, half_k:k192], ident[:])
                    nc.vector.tensor_copy(out=at2[0:half_k, :],
                                          in_=ep[0:half_k, P:2 * P])
                    nc.vector.memset(at2[half_k:half_k + 1, :], 1.0)
                    nc.tensor.matmul(ep[:, 2 * P:3 * P], at1[:], wo1[:],
                                     start=True, stop=False)
                    nc.tensor.matmul(ep[:, 2 * P:3 * P], at2[:], wo2[:],
                                     start=False, stop=True)
                    osb = outsb.tile([P, cfg.c_out], F32)
                    nc.scalar.activation(osb[:], ep[:, 2 * P:3 * P],
                                         mybir.ActivationFunctionType.Tanh)
                    nc.sync.dma_start(
                        out=out_scr[li, s * P:(s + 1) * P, :], in_=osb[:])

        if reps > 1:
            with tc.For_i(0, reps, 1):
                for li, sch in enumerate(scheds):
                    emit_layer(li, sch)
        else:
            for li, sch in enumerate(scheds):
                emit_layer(li, sch)
    nc.finalize()
    return nc


# --------------------------------------------------------------------------
# entry point
# --------------------------------------------------------------------------

def make_in_maps(cfg, x, scheds):
    x = np.asarray(x, np.float32)
    if cfg.g128:
        xp = np.zeros((cfg.n_nodes, 2 * cfg.c_in), dtype=np.float16)
        xp[:, :cfg.c_in] = x.astype(np.float16)
        tlo = np.ascontiguousarray(xp[:cfg.half])
        thi = np.ascontiguousarray(xp[cfg.half:])
    else:
        tlo = np.ascontiguousarray(x[:cfg.half])
        thi = np.ascontiguousarray(x[cfg.half:])
    in_maps = []
    for k in range(cfg.n_cores):
        m = {"x_lo": tlo, "x_hi": thi}
        for li, sch in enumerate(scheds):
            c = sch.core[k]
            m[f"idx_lo{li}"] = c["idx_lo"]
            m[f"idx_hi{li}"] = c["idx_hi"]
            m[f"attr14_{li}"] = c["attr14"]
            m[f"oneh{li}"] = c["oneh"]
            m[f"W14_{li}"] = sch.W14
            m[f"Wout1_{li}"] = sch.Wout1
            m[f"Wout2_{li}"] = sch.Wout2
        in_maps.append(m)
    return in_maps


def _run(cfg, x, layers, reps=1):
    """layers: list of (edge_index, edge_attr, W_in, b_in, W_out, b_out)."""
    scheds = [prep_layer(cfg, *lay) for lay in layers]
    nc = build_program(cfg, scheds, reps=reps)
    in_maps = make_in_maps(cfg, x, scheds)

    from concourse.bass_utils import run_bass_kernel_spmd
    res = run_bass_kernel_spmd(nc, in_maps, list(range(cfg.n_cores)),
                               trace=globals().get("TRACE", False))

    n_layers = len(layers)
    out = np.zeros((cfg.n_nodes, n_layers * cfg.c_out), dtype=np.float32)
    for k in range(cfg.n_cores):
        scr = res.results[k]["out_scr"]
        for li, sch in enumerate(scheds):
            for s in range(cfg.n_slot):
                w = sch.win_of[k, s]
                if w < 0:
                    continue
                r0 = int(w) * P
                r1 = min(r0 + P, cfg.n_nodes)
                out[r0:r1, li * cfg.c_out:(li + 1) * cfg.c_out] = \
                    scr[li, s * P:s * P + (r1 - r0), :]
    return out, res


def kernel(x, edge_index0, edge_attr0, edge_index1, edge_attr1,
           W_in0, b_in0, W_out0, b_out0, W_in1, b_in1, W_out1, b_out1):
    x = np.asarray(x)
    cfg = Cfg(n_nodes=x.shape[0], c_in=x.shape[1])
    layers = [
        (np.asarray(edge_index0), np.asarray(edge_attr0),
         np.asarray(W_in0), np.asarray(b_in0),
         np.asarray(W_out0), np.asarray(b_out0)),
        (np.asarray(edge_index1), np.asarray(edge_attr1),
         np.asarray(W_in1), np.asarray(b_in1),
         np.asarray(W_out1), np.asarray(b_out1)),
    ]
    out, _ = _run(cfg, x, layers)
    return out



# revision 16
# speedup vs baseline: 1.3928x; 1.1824x over previous
"""Trainium2 Bass kernel for nn_CFDFVGCN (two SpatialGraphConv layers, concat).

Strategy (8 NeuronCores, SPMD single program):
  - Shard by DESTINATION node windows of 128 nodes. 391 windows are balanced
    across 8 cores x 49 slots; each core aggregates its own windows entirely
    locally (no collectives).
  - Per (core, window, src-half) the edges are sorted by dst and cut into
    128-edge chunks (padded so chunk counts per slot are identical across
    cores -> one shared program).
  - Per chunk on device:
      scaling = relu([attr|1] @ [W_in;b_in])  (PE matmul, 2 chunks per matmul)
      msg     = scaling * x[src] (broadcast over hid=3)  (DVE, bf16)
      aggr   += onehot(dst_local)^T @ msg    (PE matmul into PSUM window)
    x[src] rows are fetched with dma_gather (SWDGE custom gather) from HBM.
    onehot matrices are host-precomputed fp8 and streamed in as weights.
  - Per window epilogue: transpose aggr, out = tanh(aggr @ W_out + b_out),
    DMA to a slot-indexed scratch; host unscrambles slots -> global rows.
"""

import math
import sys
from contextlib import ExitStack

import numpy as np

for _p in ("/opt/trn_rl_repo",):
    if _p not in sys.path:
        sys.path.insert(0, _p)

import ml_dtypes  # noqa: E402

import concourse.bacc as bacc  # noqa: E402
import concourse.bass as bass  # noqa: E402
import concourse.mybir as mybir  # noqa: E402
import concourse.tile as tile  # noqa: E402
from concourse.masks import make_identity  # noqa: E402

P = 128
F32 = mybir.dt.float32
F16 = mybir.dt.float16
FP8 = mybir.dt.float8e4
I16 = mybir.dt.int16

FP8_NP = mybir.dt.np(FP8)
F16_NP = np.float16


class Cfg:
    def __init__(self, n_nodes, c_in=64, hid=3, c_out=128, attr=6, n_cores=8,
                 gb=24, pb=24, eg=4, eg2=12, scratch=65536, ablate=(),
                 nq=4, aggr_bufs=2, epi_bufs=2, spsum_bufs=2, g128=True,
                 sbufs=4):
        self.n_nodes = n_nodes
        self.c_in = c_in            # 64
        self.hid = hid              # 3
        self.c_out = c_out          # 128
        self.attr = attr            # 6
        self.n_cores = n_cores
        self.half = (n_nodes + 1) // 2      # src split point for int16 gather
        self.n_win = (n_nodes + P - 1) // P
        self.n_slot = (self.n_win + n_cores - 1) // n_cores
        self.gb = gb                # chunks per gather / onehot batch
        self.pb = pb                # scaling pairs per attr batch
        self.eg = eg                # chunks per PSUM scaling group (eg/2 banks)
        self.eg2 = eg2              # chunks per elementwise supergroup
        self.sbufs = sbufs          # bufs for scal/xs/msg pools
        assert eg2 % eg == 0
        self.k192 = hid * c_in      # 192
        self.scratch = scratch
        self.ablate = set(ablate)
        self.nq = nq
        self.g128 = g128
        self.aggr_bufs = aggr_bufs
        self.epi_bufs = epi_bufs
        self.spsum_bufs = spsum_bufs
        assert (eg // 2) * spsum_bufs + aggr_bufs + epi_bufs <= 8
        # feature permutation: device feature j' = h*64+c  <->  ref j = 3c+h
        c = np.arange(c_in)
        self.perm = np.concatenate([hid * c + h for h in range(hid)])  # [192]


# --------------------------------------------------------------------------
# host-side scheduling / data prep
# --------------------------------------------------------------------------

class LayerSched:
    """Per-layer, cross-core-uniform chunk schedule + per-core data arrays."""
    pass


def _assign_windows(cfg, win_edge_counts):
    """Balance windows across cores. Returns win_of[core][slot] (-1 = pad)."""
    n_win, n_cores, n_slot = cfg.n_win, cfg.n_cores, cfg.n_slot
    order = np.argsort(-win_edge_counts, kind="stable")
    totals = np.zeros(n_cores, dtype=np.int64)
    counts = np.zeros(n_cores, dtype=np.int64)
    win_of = -np.ones((n_cores, n_slot), dtype=np.int64)
    for w in order:
        k = min((kk for kk in range(n_cores) if counts[kk] < n_slot),
                key=lambda kk: totals[kk])
        win_of[k, counts[k]] = w
        counts[k] += 1
        totals[k] += win_edge_counts[w]
    # within each core, sort slots by edge count desc so slot s across cores
    # holds comparably-sized windows (minimizes per-slot max padding)
    for k in range(n_cores):
        cnt = np.where(win_of[k] >= 0, win_edge_counts[np.maximum(win_of[k], 0)], -1)
        win_of[k] = win_of[k][np.argsort(-cnt, kind="stable")]
    return win_of


def prep_layer(cfg, edge_index, edge_attr, W_in, b_in, W_out, b_out):
    """Build the uniform schedule and per-core device arrays for one layer."""
    src = np.asarray(edge_index[0], dtype=np.int64).astype(np.int32)
    dst = np.asarray(edge_index[1], dtype=np.int64).astype(np.int32)
    attr = np.asarray(edge_attr, dtype=np.float32)
    E = src.shape[0]
    n_cores, n_slot = cfg.n_cores, cfg.n_slot

    win = dst >> 7
    if "oldassign" in cfg.ablate:
        win_counts = np.bincount(win, minlength=cfg.n_win)
        win_of = _assign_windows(cfg, win_counts)
        _skip_deal = True
    else:
        _skip_deal = False
    n_winp = cfg.n_cores * cfg.n_slot
    half_all = (src >= cfg.half)
    cl = np.bincount(win[~half_all], minlength=n_winp)
    chh = np.bincount(win[half_all], minlength=n_winp)
    clc, chc = (cl + P - 1) // P, (chh + P - 1) // P
    # deal windows sorted by chunk signature: slot s gets ranks 8s..8s+7,
    # one per core -> per-slot max over cores is tight
    if not _skip_deal:
        order = sorted(range(n_winp), key=lambda w: (int(clc[w]), int(chc[w])))
        win_of = -np.ones((cfg.n_cores, cfg.n_slot), dtype=np.int64)
        for i, w in enumerate(order):
            if w < cfg.n_win:
                win_of[i % cfg.n_cores, i // cfg.n_cores] = w

    # per (core, slot, half): edge index lists sorted by dst
    # edge order: sort once globally by (win, srchalf, dst)
    half = (src >= cfg.half).astype(np.int64)
    okey = (win.astype(np.int64) << 33) | (half << 32) | dst.astype(np.int64)
    eorder = np.argsort(okey, kind="stable")
    # boundaries per (win, half)
    wh = win.astype(np.int64) * 2 + half
    wh_sorted = wh[eorder]
    grp_start = {}
    uniq, starts = np.unique(wh_sorted, return_index=True)
    ends = np.append(starts[1:], E)
    for u, s0, e0 in zip(uniq, starts, ends):
        grp_start[int(u)] = (int(s0), int(e0))

    def group_edges(w, h):
        r = grp_start.get(int(w) * 2 + int(h))
        if r is None:
            return eorder[0:0]
        return eorder[r[0]:r[1]]

    # chunk counts per (core, slot, half)
    nch = np.zeros((n_cores, n_slot, 2), dtype=np.int64)
    for k in range(n_cores):
        for s in range(n_slot):
            w = win_of[k, s]
            if w < 0:
                continue
            for h in range(2):
                cnt = len(group_edges(w, h))
                nch[k, s, h] = (cnt + P - 1) // P
    CH = nch.max(axis=0)  # [n_slot, 2] uniform chunk counts
    for s in range(n_slot):
        if CH[s].sum() == 0:
            CH[s, 0] = 1  # every slot needs >=1 chunk to init its PSUM bank

    # global chunk schedule: per slot, lo-run then hi-run
    sched_chunks = []       # (slot, half)
    for s in range(n_slot):
        sched_chunks += [(s, 0)] * int(CH[s, 0]) + [(s, 1)] * int(CH[s, 1])
    # pad to multiple of eg2 (and so of 2) with hi-chunks on the last slot
    while len(sched_chunks) % cfg.eg2 != 0:
        sched_chunks.append((n_slot - 1, 1))
        CH[n_slot - 1, 1] += 1
    NCH = len(sched_chunks)

    # stream positions
    stream_pos = []         # per chunk: (half, pos in that stream)
    cnt_lo = cnt_hi = 0
    for (s, h) in sched_chunks:
        if h == 0:
            stream_pos.append((0, cnt_lo)); cnt_lo += 1
        else:
            stream_pos.append((1, cnt_hi)); cnt_hi += 1
    NLO, NHI = cnt_lo, cnt_hi
    NBLO = (NLO + cfg.gb - 1) // cfg.gb if NLO else 0
    NBHI = (NHI + cfg.gb - 1) // cfg.gb if NHI else 0
    NPAIR = NCH // 2
    NB14 = (NPAIR + cfg.pb - 1) // cfg.pb

    # slot boundaries: first/last chunk index per slot
    slot_first = {}
    slot_last = {}
    for ci, (s, h) in enumerate(sched_chunks):
        if s not in slot_first:
            slot_first[s] = ci
        slot_last[s] = ci

    sch = LayerSched()
    sch.cfg = cfg
    sch.win_of = win_of
    sch.CH = CH
    sch.chunks = sched_chunks
    sch.NCH = NCH
    sch.stream_pos = stream_pos
    sch.NLO, sch.NHI, sch.NBLO, sch.NBHI = NLO, NHI, NBLO, NBHI
    sch.NPAIR, sch.NB14 = NPAIR, NB14
    sch.slot_first, sch.slot_last = slot_first, slot_last

    # ---- weights (shared across cores)
    k192 = cfg.k192
    W7 = np.concatenate([np.asarray(W_in, np.float32),
                         np.asarray(b_in, np.float32)[None, :]], axis=0)  # [7,192]
    W7r = W7[:, cfg.perm]                                                 # [7,192]
    na = cfg.attr + 1
    W14 = np.zeros((2 * na, 2 * k192), dtype=np.float32)
    W14[:na, :k192] = W7r
    W14[na:, k192:] = W7r
    Woutr = np.asarray(W_out, np.float32)[cfg.perm, :]                    # [192,128]
    Wout1 = np.ascontiguousarray(Woutr[: k192 // 2])                      # [96,128]
    Wout2 = np.concatenate([Woutr[k192 // 2:],
                            np.asarray(b_out, np.float32)[None, :]], 0)   # [97,128]
    sch.W14 = W14.astype(np.float16)
    sch.Wout1 = Wout1.astype(np.float16)
    sch.Wout2 = Wout2.astype(np.float16)

    # ---- per-core arrays
    sch.core = []
    iota128 = np.arange(P, dtype=np.int32)
    for k in range(n_cores):
        # flat per-chunk edge ids (-1 = pad)
        eids = -np.ones((NCH, P), dtype=np.int64)
        for s in range(n_slot):
            w = win_of[k, s]
            if w < 0:
                continue
            for h in range(2):
                g = group_edges(w, h)
                if len(g) == 0:
                    continue
                # chunk indices for this (s,h)
                base = slot_first[s] + (0 if h == 0 else int(CH[s, 0]))
                ncs = (len(g) + P - 1) // P
                for c in range(ncs):
                    eids[base + c, : len(g[c * P:(c + 1) * P])] = g[c * P:(c + 1) * P]
        valid = eids >= 0
        e_safe = np.maximum(eids, 0)

        srcs = np.where(valid, src[e_safe], 0)
        dsts = np.where(valid, dst[e_safe], 0)

        # gather indices per stream, wrapped [16, n/16] replicated to 128 parts
        def build_idx(nb, stream_h):
            arr = np.zeros((max(nb, 1), P, cfg.gb * 8), dtype=np.int16)
            if nb == 0:
                return arr
            flat = np.zeros(nb * cfg.gb * P, dtype=np.int16)
            pos = 0
            for ci in range(NCH):
                hh, pp = stream_pos[ci]
                if hh != stream_h:
                    continue
                v = srcs[ci].astype(np.int64) - (cfg.half if stream_h else 0)
                v = np.where(valid[ci], v, 0)
                flat[pp * P:(pp + 1) * P] = v.astype(np.int16)
                pos += 1
            wrapped = flat.reshape(nb, cfg.gb * 8, 16).transpose(0, 2, 1)  # [nb,16,gb*8]
            return np.ascontiguousarray(
                np.tile(wrapped, (1, 8, 1)).astype(np.int16))              # [nb,128,gb*8]

        idx_lo = build_idx(NBLO, 0)
        idx_hi = build_idx(NBHI, 1)

        # attr14: [NB14, 14, pb*128]
        attr14 = np.zeros((NB14, 2 * na, cfg.pb * P), dtype=np.float16)
        a7 = np.zeros((NCH, na, P), dtype=np.float32)
        a7[:, :cfg.attr, :] = np.where(valid[:, None, :],
                                       attr[e_safe].transpose(0, 2, 1), 0.0)
        a7[:, cfg.attr, :] = valid.astype(np.float32)
        for pr in range(NPAIR):
            b14, off = divmod(pr, cfg.pb)
            attr14[b14, :na, off * P:(off + 1) * P] = a7[2 * pr]
            attr14[b14, na:, off * P:(off + 1) * P] = a7[2 * pr + 1]

        # onehot fp8: [128, NCH*128] partition-major (batched by gb chunks)
        slot_ids = np.array([s for (s, h) in sched_chunks], dtype=np.int64)
        dloc = dsts - (win_of[k][slot_ids][:, None] * P)
        dloc = np.where(valid, dloc, -1)
        oh = (dloc[:, :, None] == iota128[None, None, :])                 # [NCH,128,128]
        oh8 = np.where(oh, np.uint8(0x38), np.uint8(0)).transpose(1, 0, 2)
        oh8 = oh8.reshape(P, NCH * P)
        nbo = (NCH + cfg.gb - 1) // cfg.gb
        if nbo * cfg.gb * P > NCH * P:  # pad to whole onehot batches
            pad = np.zeros((P, nbo * cfg.gb * P - NCH * P), dtype=np.uint8)
            oh8 = np.concatenate([oh8, pad], axis=1)
        oh8 = np.ascontiguousarray(oh8).view(FP8_NP)

        core = {"idx_lo": idx_lo, "idx_hi": idx_hi, "attr14": attr14, "oneh": oh8}
        sch.core.append(core)
    return sch


def _dma_gather_any(g, out_ap, in_ap, idxs_ap, num_idxs, num_idxs_reg,
                    elem_size, elem_step, single_packet=False, queue_num=0):
    """dma_gather without the elem_size%256 restriction (row stride must
    still be a 256B multiple; gathers the first elem_size elems per row)."""
    stride_bytes = elem_step * mybir.dt.size(in_ap.dtype)
    assert stride_bytes % 256 == 0 and stride_bytes // 256 < 256
    _in_ap = g.lower_ap_dma(in_ap, for_custom_bir_dma=True)
    _idxs_ap = g.lower_ap(idxs_ap)
    _out_ap = g.lower_ap(out_ap)
    return g.add_instruction(mybir.InstDMAGatherAnt(
        name=g.bass.get_next_instruction_name(),
        ins=[*_in_ap, _idxs_ap, g.lower_val_access(g.to_reg(num_idxs_reg))],
        outs=[_out_ap],
        transpose=False, num_idxs=num_idxs, elem_size=elem_size,
        stride_bytes_256=stride_bytes // 256, gen_mode=0,
        single_packet=single_packet, queue_num=queue_num,
        sbuf_tokens_per_rank=0, sbuf_free_dim_per_rank=0,
        sbuf_free_dim_pad_per_rank=0, sbuf_byte_offset=0))


# --------------------------------------------------------------------------
# device program
# --------------------------------------------------------------------------

def build_program(cfg, scheds, reps=1):
    """One shared SPMD program for all cores. scheds = [layer0, layer1]."""
    nc = bacc.Bacc("TRN2", target_bir_lowering=False, debug=False,
               dynamic_dma_scratch_size=cfg.scratch,
               num_swdge_queues=cfg.nq)
    n_layers = len(scheds)
    na = cfg.attr + 1
    k192, half_k = cfg.k192, cfg.k192 // 2

    # ---- DRAM tensors (identical shapes across cores)
    if cfg.g128:
        x_lo = nc.dram_tensor("x_lo", [cfg.half, 2 * cfg.c_in], F16,
                              kind="ExternalInput")
        x_hi = nc.dram_tensor("x_hi", [cfg.n_nodes - cfg.half, 2 * cfg.c_in],
                              F16, kind="ExternalInput")
    else:
        x_lo = nc.dram_tensor("x_lo", [cfg.half, cfg.c_in], F32,
                              kind="ExternalInput")
        x_hi = nc.dram_tensor("x_hi", [cfg.n_nodes - cfg.half, cfg.c_in], F32,
                              kind="ExternalInput")
    dins = []
    for li, sch in enumerate(scheds):
        d = {}
        d["idx_lo"] = nc.dram_tensor(f"idx_lo{li}", list(sch.core[0]["idx_lo"].shape),
                                     I16, kind="ExternalInput")
        d["idx_hi"] = nc.dram_tensor(f"idx_hi{li}", list(sch.core[0]["idx_hi"].shape),
                                     I16, kind="ExternalInput")
        d["attr14"] = nc.dram_tensor(f"attr14_{li}", list(sch.core[0]["attr14"].shape),
                                     F16, kind="ExternalInput")
        d["oneh"] = nc.dram_tensor(f"oneh{li}", list(sch.core[0]["oneh"].shape),
                                   FP8, kind="ExternalInput")
        d["W14"] = nc.dram_tensor(f"W14_{li}", list(sch.W14.shape), F16,
                                  kind="ExternalInput")
        d["Wout1"] = nc.dram_tensor(f"Wout1_{li}", list(sch.Wout1.shape), F16,
                                    kind="ExternalInput")
        d["Wout2"] = nc.dram_tensor(f"Wout2_{li}", list(sch.Wout2.shape), F16,
                                    kind="ExternalInput")
        dins.append(d)
    out_scr = nc.dram_tensor("out_scr", [n_layers, cfg.n_slot * P, cfg.c_out], F32,
                             kind="ExternalOutput")

    with tile.TileContext(nc) as tc, ExitStack() as ctx:
        const = ctx.enter_context(tc.tile_pool(name="const", bufs=1))
        idxp = (None if "nogather" in cfg.ablate else
                ctx.enter_context(tc.tile_pool(name="idx", bufs=2)))
        xjp = [ctx.enter_context(tc.tile_pool(name=f"xj{h}", bufs=2)) for h in range(2)]
        attrp = ctx.enter_context(tc.tile_pool(name="attr", bufs=2))
        onehp = ctx.enter_context(tc.tile_pool(name="oneh", bufs=2))
        spsum = ctx.enter_context(tc.tile_pool(name="spsum", bufs=cfg.spsum_bufs, space="PSUM"))
        aggrp = ctx.enter_context(tc.tile_pool(name="aggr", bufs=cfg.aggr_bufs, space="PSUM"))
        epip = ctx.enter_context(tc.tile_pool(name="epip", bufs=cfg.epi_bufs, space="PSUM"))
        scalp = ctx.enter_context(tc.tile_pool(name="scal", bufs=cfg.sbufs))
        msgp = ctx.enter_context(tc.tile_pool(name="msg", bufs=cfg.sbufs))
        aggsb = ctx.enter_context(tc.tile_pool(name="aggsb", bufs=2))
        atsb = ctx.enter_context(tc.tile_pool(name="atsb", bufs=2))
        outsb = ctx.enter_context(tc.tile_pool(name="outsb", bufs=2))

        ident = const.tile([P, P], F32)
        make_identity(nc, ident[:])
        ones1 = const.tile([1, P], F16)
        nc.vector.memset(ones1[:], 1.0)

        wtiles = []
        for li, sch in enumerate(scheds):
            d = dins[li]
            w14 = const.tile([2 * na, 2 * k192], F16, tag=f"w14_{li}")
            nc.sync.dma_start(out=w14[:], in_=d["W14"][:])
            wo1 = const.tile([half_k, cfg.c_out], F16, tag=f"wo1_{li}")
            nc.sync.dma_start(out=wo1[:], in_=d["Wout1"][:])
            wo2 = const.tile([half_k, cfg.c_out], F16, tag=f"wo2_{li}")
            nc.sync.dma_start(out=wo2[:], in_=d["Wout2"][0:half_k])
            wo2b = const.tile([1, cfg.c_out], F16, tag=f"wo2b_{li}")
            nc.sync.dma_start(out=wo2b[:], in_=d["Wout2"][half_k:half_k + 1])
            wtiles.append((w14, wo1, wo2, wo2b))

        def emit_layer(li, sch):
            d = dins[li]
            w14, wo1, wo2, wo2b = wtiles[li]

            xj_tile_of = [{}, {}]        # stream -> {batch index: tile}
            oneh_tile = None
            attr_tile = None
            attr_b14 = -1
            sp_tile = None               # scaling psum group tile
            scal_tile = xs_tile = msg_tile = None
            aggr_tile = None

            def ensure_gather(stream, bi):
                if bi in xj_tile_of[stream]:
                    return
                xt = xjp[stream].tile([P, cfg.gb, cfg.c_in],
                                      F16 if cfg.g128 else F32)
                if "nogather" in cfg.ablate:
                    nc.gpsimd.memset(xt[:, 0:1, :], 0.25)
                else:
                    it = idxp.tile([P, cfg.gb * 8], I16)
                    src_dram = d["idx_lo"] if stream == 0 else d["idx_hi"]
                    nc.sync.dma_start(out=it[:], in_=src_dram[bi])
                    table = x_lo if stream == 0 else x_hi
                    if cfg.g128:
                        _dma_gather_any(
                            nc.gpsimd, out_ap=xt[:],
                            in_ap=table[:, 0:cfg.c_in], idxs_ap=it[:],
                            num_idxs=cfg.gb * P, num_idxs_reg=cfg.gb * P,
                            elem_size=cfg.c_in, elem_step=2 * cfg.c_in,
                            single_packet=False,
                            queue_num=(stream * 7 + bi) % cfg.nq)
                    else:
                        nc.gpsimd.dma_gather(
                            out_ap=xt[:], in_ap=table[:], idxs_ap=it[:],
                            num_idxs=cfg.gb * P, num_idxs_reg=cfg.gb * P,
                            elem_size=cfg.c_in, single_packet=False,
                            queue_num=(stream * 7 + bi) % cfg.nq)
                xj_tile_of[stream][bi] = xt

            for ci in range(sch.NCH):
                s, h = sch.chunks[ci]

                # ---- supergroup head: gathers, scaling, relu, stage, mul
                if ci % cfg.eg2 == 0:
                    for cj in range(ci, ci + cfg.eg2):
                        st_j, pos_j = sch.stream_pos[cj]
                        ensure_gather(st_j, pos_j // cfg.gb)
                    scal_tile = scalp.tile([P, cfg.eg2, k192], F16)
                    for g0 in range(0, cfg.eg2, cfg.eg):
                        sp_tile = spsum.tile([P, cfg.eg // 2, 512], F32)
                        for pj in range(cfg.eg // 2):
                            pr = (ci + g0) // 2 + pj
                            b14, poff = divmod(pr, cfg.pb)
                            if b14 != attr_b14:
                                attr_tile = attrp.tile([2 * na, cfg.pb * P],
                                                       F16)
                                nc.sync.dma_start(out=attr_tile[:],
                                                  in_=d["attr14"][b14])
                                attr_b14 = b14
                            if "nosc" in cfg.ablate:
                                nc.vector.memset(sp_tile[:, pj, 0:8], 0.0)
                            else:
                                nc.tensor.matmul(
                                    sp_tile[:, pj, 0:2 * k192],
                                    attr_tile[:, poff * P:(poff + 1) * P],
                                    w14[:],
                                    start=True, stop=True)
                        if "noelem" in cfg.ablate:
                            nc.gpsimd.memset(
                                scal_tile[:, g0:g0 + 1, 0:8], 0.25)
                        else:
                            nc.scalar.activation(
                                scal_tile[:, g0:g0 + cfg.eg, :],
                                sp_tile[:, :, 0:2 * k192],
                                mybir.ActivationFunctionType.Relu)
                    # msg = scal * xj, read directly from the gather tiles per
                    # contiguous run. tensor_tensor only runs in 1-port DVE
                    # mode — 2-port DVE ops (copy/cast/memset) fully block
                    # SWDGE descriptor generation and starve the gathers.
                    msg_tile = msgp.tile([P, cfg.eg2, k192], F16)
                    if "noelem" in cfg.ablate:
                        nc.gpsimd.memset(msg_tile[:, 0:1, 0:8], 0.25)
                    j = 0
                    while j < cfg.eg2 and "noelem" not in cfg.ablate:
                        st_j, pos_j = sch.stream_pos[ci + j]
                        bi_j, off_j = divmod(pos_j, cfg.gb)
                        run = 1
                        while (j + run < cfg.eg2):
                            st_n, pos_n = sch.stream_pos[ci + j + run]
                            if st_n != st_j or pos_n != pos_j + run:
                                break
                            if divmod(pos_n, cfg.gb)[0] != bi_j:
                                break
                            run += 1
                        xsl = xj_tile_of[st_j][bi_j][:, off_j:off_j + run, :]
                        for hh in range(cfg.hid):
                            o0 = hh * cfg.c_in
                            nc.vector.tensor_tensor(
                                out=msg_tile[:, j:j + run, o0:o0 + cfg.c_in],
                                in0=scal_tile[:, j:j + run, o0:o0 + cfg.c_in],
                                in1=xsl,
                                op=mybir.AluOpType.mult)
                        j += run

                # ---- onehot batch load
                obi, ooff = divmod(ci, cfg.gb)
                if ooff == 0:
                    oneh_tile = onehp.tile([P, cfg.gb * P], FP8)
                    nc.sync.dma_start(
                        out=oneh_tile[:],
                        in_=d["oneh"][:, obi * cfg.gb * P:(obi + 1) * cfg.gb * P])

                eoff = ci % cfg.eg2
                # ---- scatter matmul into the slot's aggr window
                if "noscatter" in cfg.ablate:
                    continue
                if ci == sch.slot_first[s]:
                    aggr_tile = aggrp.tile([P, 512], F32)
                nc.tensor.matmul(
                    aggr_tile[:, 0:k192],
                    oneh_tile[:, ooff * P:(ooff + 1) * P],
                    msg_tile[:, eoff, :],
                    start=(ci == sch.slot_first[s]),
                    stop=(ci == sch.slot_last[s]))

                # ---- epilogue at slot end
                if ci == sch.slot_last[s]:
                    asb = aggsb.tile([P, k192], F32)
                    nc.scalar.activation(asb[:], aggr_tile[:, 0:k192],
                                         mybir.ActivationFunctionType.Copy)
                    ep = epip.tile([P, 512], F32)
                    at1 = atsb.tile([half_k, P], F16, tag="at1")
                    at2 = atsb.tile([half_k, P], F16, tag="at2")
                    nc.tensor.transpose(ep[0:half_k, 0:P], asb[:, 0:half_k],
                                        ident[:])
                    nc.scalar.activation(at1[:], ep[0:half_k, 0:P],
                                         mybir.ActivationFunctionType.Copy)
                    nc.tensor.transpose(ep[0:half_k, P:2 * P],
                                        asb[:, half_k:k192], ident[:])
                    nc.scalar.activation(at2[:], ep[0:half_k, P:2 * P],
                                         mybir.ActivationFunctionType.Copy)
                    nc.tensor.matmul(ep[:, 2 * P:3 * P], at1[:], wo1[:],
                                     start=True, stop=False)
                    nc.tensor.matmul(ep[:, 2 * P:3 * P], at2[:],
                                     wo2[:], start=False, stop=False)
                    nc.tensor.matmul(ep[:, 2 * P:3 * P], ones1[:],
                                     wo2b[:], start=False, stop=True)
                    osb = outsb.tile([P, cfg.c_out], F32)
                    nc.scalar.activation(osb[:], ep[:, 2 * P:3 * P],
                                         mybir.ActivationFunctionType.Tanh)
                    nc.sync.dma_start(
                        out=out_scr[li, s * P:(s + 1) * P, :], in_=osb[:])

        if reps > 1:
            with tc.For_i(0, reps, 1):
                for li, sch in enumerate(scheds):
                    emit_layer(li, sch)
        else:
            for li, sch in enumerate(scheds):
                emit_layer(li, sch)
    nc.finalize()
    return nc


# --------------------------------------------------------------------------
# entry point
# --------------------------------------------------------------------------

def make_in_maps(cfg, x, scheds):
    x = np.asarray(x, np.float32)
    if cfg.g128:
        xp = np.zeros((cfg.n_nodes, 2 * cfg.c_in), dtype=np.float16)
        xp[:, :cfg.c_in] = x.astype(np.float16)
        tlo = np.ascontiguousarray(xp[:cfg.half])
        thi = np.ascontiguousarray(xp[cfg.half:])
    else:
        tlo = np.ascontiguousarray(x[:cfg.half])
        thi = np.ascontiguousarray(x[cfg.half:])
    in_maps = []
    for k in range(cfg.n_cores):
        m = {"x_lo": tlo, "x_hi": thi}
        for li, sch in enumerate(scheds):
            c = sch.core[k]
            m[f"idx_lo{li}"] = c["idx_lo"]
            m[f"idx_hi{li}"] = c["idx_hi"]
            m[f"attr14_{li}"] = c["attr14"]
            m[f"oneh{li}"] = c["oneh"]
            m[f"W14_{li}"] = sch.W14
            m[f"Wout1_{li}"] = sch.Wout1
            m[f"Wout2_{li}"] = sch.Wout2
        in_maps.append(m)
    return in_maps


def _run(cfg, x, layers, reps=1):
    """layers: list of (edge_index, edge_attr, W_in, b_in, W_out, b_out)."""
    scheds = [prep_layer(cfg, *lay) for lay in layers]
    nc = build_program(cfg, scheds, reps=reps)
    in_maps = make_in_maps(cfg, x, scheds)

    from concourse.bass_utils import run_bass_kernel_spmd
    res = run_bass_kernel_spmd(nc, in_maps, list(range(cfg.n_cores)),
                               trace=globals().get("TRACE", False))

    n_layers = len(layers)
    out = np.zeros((cfg.n_nodes, n_layers * cfg.c_out), dtype=np.float32)
    for k in range(cfg.n_cores):
        scr = res.results[k]["out_scr"]
        for li, sch in enumerate(scheds):
            for s in range(cfg.n_slot):
                w = sch.win_of[k, s]
                if w < 0:
                    continue
                r0 = int(w) * P
                r1 = min(r0 + P, cfg.n_nodes)
                out[r0:r1, li * cfg.c_out:(li + 1) * cfg.c_out] = \
                    scr[li, s * P:s * P + (r1 - r0), :]
    return out, res


def kernel(x, edge_index0, edge_attr0, edge_index1, edge_attr1,
           W_in0, b_in0, W_out0, b_out0, W_in1, b_in1, W_out1, b_out1):
    x = np.asarray(x)
    cfg = Cfg(n_nodes=x.shape[0], c_in=x.shape[1])
    layers = [
        (np.asarray(edge_index0), np.asarray(edge_attr0),
         np.asarray(W_in0), np.asarray(b_in0),
         np.asarray(W_out0), np.asarray(b_out0)),
        (np.asarray(edge_index1), np.asarray(edge_attr1),
         np.asarray(W_in1), np.asarray(b_in1),
         np.asarray(W_out1), np.asarray(b_out1)),
    ]
    out, _ = _run(cfg, x, layers)
    return out



# revision 19
# speedup vs baseline: 1.4215x; 1.0206x over previous
"""Trainium2 Bass kernel for nn_CFDFVGCN (two SpatialGraphConv layers, concat).

Strategy (8 NeuronCores, SPMD single program):
  - Shard by DESTINATION node windows of 128 nodes. 391 windows are balanced
    across 8 cores x 49 slots; each core aggregates its own windows entirely
    locally (no collectives).
  - Per (core, window, src-half) the edges are sorted by dst and cut into
    128-edge chunks (padded so chunk counts per slot are identical across
    cores -> one shared program).
  - Per chunk on device:
      scaling = relu([attr|1] @ [W_in;b_in])  (PE matmul, 2 chunks per matmul)
      msg     = scaling * x[src] (broadcast over hid=3)  (DVE, bf16)
      aggr   += onehot(dst_local)^T @ msg    (PE matmul into PSUM window)
    x[src] rows are fetched with dma_gather (SWDGE custom gather) from HBM.
    onehot matrices are host-precomputed fp8 and streamed in as weights.
  - Per window epilogue: transpose aggr, out = tanh(aggr @ W_out + b_out),
    DMA to a slot-indexed scratch; host unscrambles slots -> global rows.
"""

import math
import sys
from contextlib import ExitStack

import numpy as np

for _p in ("/opt/trn_rl_repo",):
    if _p not in sys.path:
        sys.path.insert(0, _p)

import ml_dtypes  # noqa: E402

import concourse.bacc as bacc  # noqa: E402
import concourse.bass as bass  # noqa: E402
import concourse.mybir as mybir  # noqa: E402
import concourse.tile as tile  # noqa: E402
from concourse.masks import make_identity  # noqa: E402

P = 128
F32 = mybir.dt.float32
F16 = mybir.dt.float16
FP8 = mybir.dt.float8e4
I16 = mybir.dt.int16

FP8_NP = mybir.dt.np(FP8)
F16_NP = np.float16


class Cfg:
    def __init__(self, n_nodes, c_in=64, hid=3, c_out=128, attr=6, n_cores=8,
                 gb=24, pb=24, eg=4, eg2=12, scratch=65536, ablate=(),
                 nq=4, aggr_bufs=2, epi_bufs=2, spsum_bufs=2, g128=True,
                 sbufs=4):
        self.n_nodes = n_nodes
        self.c_in = c_in            # 64
        self.hid = hid              # 3
        self.c_out = c_out          # 128
        self.attr = attr            # 6
        self.n_cores = n_cores
        self.half = (n_nodes + 1) // 2      # src split point for int16 gather
        self.n_win = (n_nodes + P - 1) // P
        self.n_slot = (self.n_win + n_cores - 1) // n_cores
        self.gb = gb                # chunks per gather / onehot batch
        self.pb = pb                # scaling pairs per attr batch
        self.eg = eg                # chunks per PSUM scaling group (eg/2 banks)
        self.eg2 = eg2              # chunks per elementwise supergroup
        self.sbufs = sbufs          # bufs for scal/xs/msg pools
        assert eg2 % eg == 0
        self.k192 = hid * c_in      # 192
        self.scratch = scratch
        self.ablate = set(ablate)
        self.nq = nq
        self.g128 = g128
        self.aggr_bufs = aggr_bufs
        self.epi_bufs = epi_bufs
        self.spsum_bufs = spsum_bufs
        assert (eg // 2) * spsum_bufs + aggr_bufs + epi_bufs <= 8
        # feature permutation: device feature j' = h*64+c  <->  ref j = 3c+h
        c = np.arange(c_in)
        self.perm = np.concatenate([hid * c + h for h in range(hid)])  # [192]


# --------------------------------------------------------------------------
# host-side scheduling / data prep
# --------------------------------------------------------------------------

class LayerSched:
    """Per-layer, cross-core-uniform chunk schedule + per-core data arrays."""
    pass


def _assign_windows(cfg, win_edge_counts):
    """Balance windows across cores. Returns win_of[core][slot] (-1 = pad)."""
    n_win, n_cores, n_slot = cfg.n_win, cfg.n_cores, cfg.n_slot
    order = np.argsort(-win_edge_counts, kind="stable")
    totals = np.zeros(n_cores, dtype=np.int64)
    counts = np.zeros(n_cores, dtype=np.int64)
    win_of = -np.ones((n_cores, n_slot), dtype=np.int64)
    for w in order:
        k = min((kk for kk in range(n_cores) if counts[kk] < n_slot),
                key=lambda kk: totals[kk])
        win_of[k, counts[k]] = w
        counts[k] += 1
        totals[k] += win_edge_counts[w]
    # within each core, sort slots by edge count desc so slot s across cores
    # holds comparably-sized windows (minimizes per-slot max padding)
    for k in range(n_cores):
        cnt = np.where(win_of[k] >= 0, win_edge_counts[np.maximum(win_of[k], 0)], -1)
        win_of[k] = win_of[k][np.argsort(-cnt, kind="stable")]
    return win_of


def prep_layer(cfg, edge_index, edge_attr, W_in, b_in, W_out, b_out):
    """Build the uniform schedule and per-core device arrays for one layer."""
    src = np.asarray(edge_index[0], dtype=np.int64).astype(np.int32)
    dst = np.asarray(edge_index[1], dtype=np.int64).astype(np.int32)
    attr = np.asarray(edge_attr, dtype=np.float32)
    E = src.shape[0]
    n_cores, n_slot = cfg.n_cores, cfg.n_slot

    win = dst >> 7
    if "oldassign" in cfg.ablate:
        win_counts = np.bincount(win, minlength=cfg.n_win)
        win_of = _assign_windows(cfg, win_counts)
        _skip_deal = True
    else:
        _skip_deal = False
    n_winp = cfg.n_cores * cfg.n_slot
    half_all = (src >= cfg.half)
    cl = np.bincount(win[~half_all], minlength=n_winp)
    chh = np.bincount(win[half_all], minlength=n_winp)
    clc, chc = (cl + P - 1) // P, (chh + P - 1) // P
    # deal windows sorted by chunk signature: slot s gets ranks 8s..8s+7,
    # one per core -> per-slot max over cores is tight
    if not _skip_deal:
        order = sorted(range(n_winp), key=lambda w: (int(clc[w]), int(chc[w])))
        win_of = -np.ones((cfg.n_cores, cfg.n_slot), dtype=np.int64)
        for i, w in enumerate(order):
            if w < cfg.n_win:
                win_of[i % cfg.n_cores, i // cfg.n_cores] = w

    # per (core, slot, half): edge index lists sorted by dst
    # edge order: sort once globally by (win, srchalf, dst)
    half = (src >= cfg.half).astype(np.int64)
    okey = (win.astype(np.int64) << 33) | (half << 32) | dst.astype(np.int64)
    eorder = np.argsort(okey, kind="stable")
    # boundaries per (win, half)
    wh = win.astype(np.int64) * 2 + half
    wh_sorted = wh[eorder]
    grp_start = {}
    uniq, starts = np.unique(wh_sorted, return_index=True)
    ends = np.append(starts[1:], E)
    for u, s0, e0 in zip(uniq, starts, ends):
        grp_start[int(u)] = (int(s0), int(e0))

    def group_edges(w, h):
        r = grp_start.get(int(w) * 2 + int(h))
        if r is None:
            return eorder[0:0]
        return eorder[r[0]:r[1]]

    # chunk counts per (core, slot, half)
    nch = np.zeros((n_cores, n_slot, 2), dtype=np.int64)
    for k in range(n_cores):
        for s in range(n_slot):
            w = win_of[k, s]
            if w < 0:
                continue
            for h in range(2):
                cnt = len(group_edges(w, h))
                nch[k, s, h] = (cnt + P - 1) // P
    CH = nch.max(axis=0)  # [n_slot, 2] uniform chunk counts
    for s in range(n_slot):
        if CH[s].sum() == 0:
            CH[s, 0] = 1  # every slot needs >=1 chunk to init its PSUM bank

    # global chunk schedule: per slot, lo-run then hi-run
    sched_chunks = []       # (slot, half)
    for s in range(n_slot):
        sched_chunks += [(s, 0)] * int(CH[s, 0]) + [(s, 1)] * int(CH[s, 1])
    # pad to multiple of eg2 (and so of 2) with hi-chunks on the last slot
    while len(sched_chunks) % cfg.eg2 != 0:
        sched_chunks.append((n_slot - 1, 1))
        CH[n_slot - 1, 1] += 1
    NCH = len(sched_chunks)

    # stream positions
    stream_pos = []         # per chunk: (half, pos in that stream)
    cnt_lo = cnt_hi = 0
    for (s, h) in sched_chunks:
        if h == 0:
            stream_pos.append((0, cnt_lo)); cnt_lo += 1
        else:
            stream_pos.append((1, cnt_hi)); cnt_hi += 1
    NLO, NHI = cnt_lo, cnt_hi
    NBLO = (NLO + cfg.gb - 1) // cfg.gb if NLO else 0
    NBHI = (NHI + cfg.gb - 1) // cfg.gb if NHI else 0
    NPAIR = NCH // 2
    NB14 = (NPAIR + cfg.pb - 1) // cfg.pb

    # slot boundaries: first/last chunk index per slot
    slot_first = {}
    slot_last = {}
    for ci, (s, h) in enumerate(sched_chunks):
        if s not in slot_first:
            slot_first[s] = ci
        slot_last[s] = ci

    sch = LayerSched()
    sch.cfg = cfg
    sch.win_of = win_of
    sch.CH = CH
    sch.chunks = sched_chunks
    sch.NCH = NCH
    sch.stream_pos = stream_pos
    sch.NLO, sch.NHI, sch.NBLO, sch.NBHI = NLO, NHI, NBLO, NBHI
    sch.NPAIR, sch.NB14 = NPAIR, NB14
    sch.slot_first, sch.slot_last = slot_first, slot_last

    # ---- weights (shared across cores)
    k192 = cfg.k192
    W7 = np.concatenate([np.asarray(W_in, np.float32),
                         np.asarray(b_in, np.float32)[None, :]], axis=0)  # [7,192]
    W7r = W7[:, cfg.perm]                                                 # [7,192]
    na = cfg.attr + 1
    W14 = np.zeros((2 * na, 2 * k192), dtype=np.float32)
    W14[:na, :k192] = W7r
    W14[na:, k192:] = W7r
    Woutr = np.asarray(W_out, np.float32)[cfg.perm, :]                    # [192,128]
    Wout1 = np.ascontiguousarray(Woutr[: k192 // 2])                      # [96,128]
    Wout2 = np.concatenate([Woutr[k192 // 2:],
                            np.asarray(b_out, np.float32)[None, :]], 0)   # [97,128]
    sch.W14 = W14.astype(np.float16)
    sch.Wout1 = Wout1.astype(np.float16)
    sch.Wout2 = Wout2.astype(np.float16)

    # ---- per-core arrays
    sch.core = []
    iota128 = np.arange(P, dtype=np.int32)
    for k in range(n_cores):
        # flat per-chunk edge ids (-1 = pad)
        eids = -np.ones((NCH, P), dtype=np.int64)
        for s in range(n_slot):
            w = win_of[k, s]
            if w < 0:
                continue
            for h in range(2):
                g = group_edges(w, h)
                if len(g) == 0:
                    continue
                # chunk indices for this (s,h)
                base = slot_first[s] + (0 if h == 0 else int(CH[s, 0]))
                ncs = (len(g) + P - 1) // P
                for c in range(ncs):
                    eids[base + c, : len(g[c * P:(c + 1) * P])] = g[c * P:(c + 1) * P]
        valid = eids >= 0
        e_safe = np.maximum(eids, 0)

        srcs = np.where(valid, src[e_safe], 0)
        dsts = np.where(valid, dst[e_safe], 0)

        # gather indices per stream, wrapped [16, n/16] replicated to 128 parts
        def build_idx(nb, stream_h):
            arr = np.zeros((max(nb, 1), P, cfg.gb * 8), dtype=np.int16)
            if nb == 0:
                return arr
            flat = np.zeros(nb * cfg.gb * P, dtype=np.int16)
            pos = 0
            for ci in range(NCH):
                hh, pp = stream_pos[ci]
                if hh != stream_h:
                    continue
                v = srcs[ci].astype(np.int64) - (cfg.half if stream_h else 0)
                v = np.where(valid[ci], v, 0)
                flat[pp * P:(pp + 1) * P] = v.astype(np.int16)
                pos += 1
            wrapped = flat.reshape(nb, cfg.gb * 8, 16).transpose(0, 2, 1)  # [nb,16,gb*8]
            return np.ascontiguousarray(
                np.tile(wrapped, (1, 8, 1)).astype(np.int16))              # [nb,128,gb*8]

        idx_lo = build_idx(NBLO, 0)
        idx_hi = build_idx(NBHI, 1)

        # attr14: [NB14, 14, pb*128]
        attr14 = np.zeros((NB14, 2 * na, cfg.pb * P), dtype=np.float16)
        a7 = np.zeros((NCH, na, P), dtype=np.float32)
        a7[:, :cfg.attr, :] = np.where(valid[:, None, :],
                                       attr[e_safe].transpose(0, 2, 1), 0.0)
        a7[:, cfg.attr, :] = valid.astype(np.float32)
        for pr in range(NPAIR):
            b14, off = divmod(pr, cfg.pb)
            attr14[b14, :na, off * P:(off + 1) * P] = a7[2 * pr]
            attr14[b14, na:, off * P:(off + 1) * P] = a7[2 * pr + 1]

        # onehot fp8: [128, NCH*128] partition-major (batched by gb chunks)
        slot_ids = np.array([s for (s, h) in sched_chunks], dtype=np.int64)
        dloc = dsts - (win_of[k][slot_ids][:, None] * P)
        dloc = np.where(valid, dloc, -1)
        oh = (dloc[:, :, None] == iota128[None, None, :])                 # [NCH,128,128]
        oh8 = np.where(oh, np.uint8(0x38), np.uint8(0)).transpose(1, 0, 2)
        oh8 = oh8.reshape(P, NCH * P)
        nbo = (NCH + cfg.gb - 1) // cfg.gb
        if nbo * cfg.gb * P > NCH * P:  # pad to whole onehot batches
            pad = np.zeros((P, nbo * cfg.gb * P - NCH * P), dtype=np.uint8)
            oh8 = np.concatenate([oh8, pad], axis=1)
        oh8 = np.ascontiguousarray(oh8).view(FP8_NP)

        core = {"idx_lo": idx_lo, "idx_hi": idx_hi, "attr14": attr14, "oneh": oh8}
        sch.core.append(core)
    return sch


def _dma_gather_any(g, out_ap, in_ap, idxs_ap, num_idxs, num_idxs_reg,
                    elem_size, elem_step, single_packet=False, queue_num=0):
    """dma_gather without the elem_size%256 restriction (row stride must
    still be a 256B multiple; gathers the first elem_size elems per row)."""
    stride_bytes = elem_step * mybir.dt.size(in_ap.dtype)
    assert stride_bytes % 256 == 0 and stride_bytes // 256 < 256
    _in_ap = g.lower_ap_dma(in_ap, for_custom_bir_dma=True)
    _idxs_ap = g.lower_ap(idxs_ap)
    _out_ap = g.lower_ap(out_ap)
    return g.add_instruction(mybir.InstDMAGatherAnt(
        name=g.bass.get_next_instruction_name(),
        ins=[*_in_ap, _idxs_ap, g.lower_val_access(g.to_reg(num_idxs_reg))],
        outs=[_out_ap],
        transpose=False, num_idxs=num_idxs, elem_size=elem_size,
        stride_bytes_256=stride_bytes // 256, gen_mode=0,
        single_packet=single_packet, queue_num=queue_num,
        sbuf_tokens_per_rank=0, sbuf_free_dim_per_rank=0,
        sbuf_free_dim_pad_per_rank=0, sbuf_byte_offset=0))


# --------------------------------------------------------------------------
# device program
# --------------------------------------------------------------------------

def build_program(cfg, scheds, reps=1):
    """One shared SPMD program for all cores. scheds = [layer0, layer1]."""
    nc = bacc.Bacc("TRN2", target_bir_lowering=False, debug=False,
               dynamic_dma_scratch_size=cfg.scratch,
               num_swdge_queues=cfg.nq)
    n_layers = len(scheds)
    na = cfg.attr + 1
    k192, half_k = cfg.k192, cfg.k192 // 2

    # ---- DRAM tensors (identical shapes across cores)
    if cfg.g128:
        x_lo = nc.dram_tensor("x_lo", [cfg.half, 2 * cfg.c_in], F16,
                              kind="ExternalInput")
        x_hi = nc.dram_tensor("x_hi", [cfg.n_nodes - cfg.half, 2 * cfg.c_in],
                              F16, kind="ExternalInput")
    else:
        x_lo = nc.dram_tensor("x_lo", [cfg.half, cfg.c_in], F32,
                              kind="ExternalInput")
        x_hi = nc.dram_tensor("x_hi", [cfg.n_nodes - cfg.half, cfg.c_in], F32,
                              kind="ExternalInput")
    dins = []
    for li, sch in enumerate(scheds):
        d = {}
        d["idx_lo"] = nc.dram_tensor(f"idx_lo{li}", list(sch.core[0]["idx_lo"].shape),
                                     I16, kind="ExternalInput")
        d["idx_hi"] = nc.dram_tensor(f"idx_hi{li}", list(sch.core[0]["idx_hi"].shape),
                                     I16, kind="ExternalInput")
        d["attr14"] = nc.dram_tensor(f"attr14_{li}", list(sch.core[0]["attr14"].shape),
                                     F16, kind="ExternalInput")
        d["oneh"] = nc.dram_tensor(f"oneh{li}", list(sch.core[0]["oneh"].shape),
                                   FP8, kind="ExternalInput")
        d["W14"] = nc.dram_tensor(f"W14_{li}", list(sch.W14.shape), F16,
                                  kind="ExternalInput")
        d["Wout1"] = nc.dram_tensor(f"Wout1_{li}", list(sch.Wout1.shape), F16,
                                    kind="ExternalInput")
        d["Wout2"] = nc.dram_tensor(f"Wout2_{li}", list(sch.Wout2.shape), F16,
                                    kind="ExternalInput")
        dins.append(d)
    out_scr = nc.dram_tensor("out_scr", [n_layers, cfg.n_slot * P, cfg.c_out], F32,
                             kind="ExternalOutput")

    with tile.TileContext(nc) as tc, ExitStack() as ctx:
        const = ctx.enter_context(tc.tile_pool(name="const", bufs=1))
        idxp = (None if "nogather" in cfg.ablate else
                ctx.enter_context(tc.tile_pool(name="idx", bufs=2)))
        xjp = [ctx.enter_context(tc.tile_pool(name=f"xj{h}", bufs=2)) for h in range(2)]
        attrp = ctx.enter_context(tc.tile_pool(name="attr", bufs=2))
        onehp = ctx.enter_context(tc.tile_pool(name="oneh", bufs=2))
        spsum = ctx.enter_context(tc.tile_pool(name="spsum", bufs=cfg.spsum_bufs, space="PSUM"))
        aggrp = ctx.enter_context(tc.tile_pool(name="aggr", bufs=cfg.aggr_bufs, space="PSUM"))
        epip = ctx.enter_context(tc.tile_pool(name="epip", bufs=cfg.epi_bufs, space="PSUM"))
        scalp = ctx.enter_context(tc.tile_pool(name="scal", bufs=cfg.sbufs))
        msgp = ctx.enter_context(tc.tile_pool(name="msg", bufs=cfg.sbufs))
        aggsb = ctx.enter_context(tc.tile_pool(name="aggsb", bufs=2))
        atsb = ctx.enter_context(tc.tile_pool(name="atsb", bufs=2))
        outsb = ctx.enter_context(tc.tile_pool(name="outsb", bufs=2))

        ident = const.tile([P, P], F32)
        make_identity(nc, ident[:])
        ones1 = const.tile([1, P], F16)
        nc.vector.memset(ones1[:], 1.0)

        wtiles = []
        for li, sch in enumerate(scheds):
            d = dins[li]
            w14 = const.tile([2 * na, 2 * k192], F16, tag=f"w14_{li}")
            nc.sync.dma_start(out=w14[:], in_=d["W14"][:])
            wo1 = const.tile([half_k, cfg.c_out], F16, tag=f"wo1_{li}")
            nc.sync.dma_start(out=wo1[:], in_=d["Wout1"][:])
            wo2 = const.tile([half_k, cfg.c_out], F16, tag=f"wo2_{li}")
            nc.sync.dma_start(out=wo2[:], in_=d["Wout2"][0:half_k])
            wo2b = const.tile([1, cfg.c_out], F16, tag=f"wo2b_{li}")
            nc.sync.dma_start(out=wo2b[:], in_=d["Wout2"][half_k:half_k + 1])
            wtiles.append((w14, wo1, wo2, wo2b))

        def emit_layer(li, sch):
            d = dins[li]
            w14, wo1, wo2, wo2b = wtiles[li]

            xj_tile_of = [{}, {}]        # stream -> {batch index: tile}
            oneh_tile = None
            attr_tile = None
            attr_b14 = -1
            sp_tile = None               # scaling psum group tile
            scal_tile = xs_tile = msg_tile = None
            aggr_tile = None

            def ensure_gather(stream, bi):
                if bi in xj_tile_of[stream]:
                    return
                xt = xjp[stream].tile([P, cfg.gb, cfg.c_in],
                                      F16 if cfg.g128 else F32)
                if "nogather" in cfg.ablate:
                    nc.gpsimd.memset(xt[:, 0:1, :], 0.25)
                else:
                    it = idxp.tile([P, cfg.gb * 8], I16)
                    src_dram = d["idx_lo"] if stream == 0 else d["idx_hi"]
                    nc.sync.dma_start(out=it[:], in_=src_dram[bi])
                    table = x_lo if stream == 0 else x_hi
                    if cfg.g128:
                        _dma_gather_any(
                            nc.gpsimd, out_ap=xt[:],
                            in_ap=table[:, 0:cfg.c_in], idxs_ap=it[:],
                            num_idxs=cfg.gb * P, num_idxs_reg=cfg.gb * P,
                            elem_size=cfg.c_in, elem_step=2 * cfg.c_in,
                            single_packet=False,
                            queue_num=(stream * 7 + bi) % cfg.nq)
                    else:
                        nc.gpsimd.dma_gather(
                            out_ap=xt[:], in_ap=table[:], idxs_ap=it[:],
                            num_idxs=cfg.gb * P, num_idxs_reg=cfg.gb * P,
                            elem_size=cfg.c_in, single_packet=False,
                            queue_num=(stream * 7 + bi) % cfg.nq)
                xj_tile_of[stream][bi] = xt

            for ci in range(sch.NCH):
                s, h = sch.chunks[ci]

                # ---- supergroup head: gathers, scaling, relu, stage, mul
                if ci % cfg.eg2 == 0:
                    for cj in range(ci, ci + cfg.eg2):
                        st_j, pos_j = sch.stream_pos[cj]
                        ensure_gather(st_j, pos_j // cfg.gb)
                    scal_tile = scalp.tile([P, cfg.eg2, k192], F16)
                    for g0 in range(0, cfg.eg2, cfg.eg):
                        sp_tile = spsum.tile([P, cfg.eg // 2, 512], F32)
                        for pj in range(cfg.eg // 2):
                            pr = (ci + g0) // 2 + pj
                            b14, poff = divmod(pr, cfg.pb)
                            if b14 != attr_b14:
                                attr_tile = attrp.tile([2 * na, cfg.pb * P],
                                                       F16)
                                nc.sync.dma_start(out=attr_tile[:],
                                                  in_=d["attr14"][b14])
                                attr_b14 = b14
                            if "nosc" in cfg.ablate:
                                nc.vector.memset(sp_tile[:, pj, 0:8], 0.0)
                            else:
                                nc.tensor.matmul(
                                    sp_tile[:, pj, 0:2 * k192],
                                    attr_tile[:, poff * P:(poff + 1) * P],
                                    w14[:],
                                    start=True, stop=True)
                        if "noelem" in cfg.ablate:
                            nc.gpsimd.memset(
                                scal_tile[:, g0:g0 + 1, 0:8], 0.25)
                        else:
                            nc.scalar.activation(
                                scal_tile[:, g0:g0 + cfg.eg, :],
                                sp_tile[:, :, 0:2 * k192],
                                mybir.ActivationFunctionType.Relu)
                    # msg = scal * xj, read directly from the gather tiles per
                    # contiguous run. tensor_tensor only runs in 1-port DVE
                    # mode — 2-port DVE ops (copy/cast/memset) fully block
                    # SWDGE descriptor generation and starve the gathers.
                    msg_tile = msgp.tile([P, cfg.eg2, k192], F16)
                    if "noelem" in cfg.ablate:
                        nc.gpsimd.memset(msg_tile[:, 0:1, 0:8], 0.25)
                    j = 0
                    while j < cfg.eg2 and "noelem" not in cfg.ablate:
                        st_j, pos_j = sch.stream_pos[ci + j]
                        bi_j, off_j = divmod(pos_j, cfg.gb)
                        run = 1
                        while (j + run < cfg.eg2):
                            st_n, pos_n = sch.stream_pos[ci + j + run]
                            if st_n != st_j or pos_n != pos_j + run:
                                break
                            if divmod(pos_n, cfg.gb)[0] != bi_j:
                                break
                            run += 1
                        xsl = xj_tile_of[st_j][bi_j][:, off_j:off_j + run, :]
                        for hh in range(cfg.hid):
                            o0 = hh * cfg.c_in
                            nc.vector.tensor_tensor(
                                out=msg_tile[:, j:j + run, o0:o0 + cfg.c_in],
                                in0=scal_tile[:, j:j + run, o0:o0 + cfg.c_in],
                                in1=xsl,
                                op=mybir.AluOpType.mult)
                        j += run

                # ---- onehot batch load
                obi, ooff = divmod(ci, cfg.gb)
                if ooff == 0:
                    oneh_tile = onehp.tile([P, cfg.gb * P], FP8)
                    # scalar (ACT) HWDGE ring — keeps the big onehot stream
                    # off the SP ring that serves attr/idx/out
                    nc.scalar.dma_start(
                        out=oneh_tile[:],
                        in_=d["oneh"][:, obi * cfg.gb * P:(obi + 1) * cfg.gb * P])

                eoff = ci % cfg.eg2
                # ---- scatter matmul into the slot's aggr window
                if "noscatter" in cfg.ablate:
                    continue
                if ci == sch.slot_first[s]:
                    aggr_tile = aggrp.tile([P, 512], F32)
                nc.tensor.matmul(
                    aggr_tile[:, 0:k192],
                    oneh_tile[:, ooff * P:(ooff + 1) * P],
                    msg_tile[:, eoff, :],
                    start=(ci == sch.slot_first[s]),
                    stop=(ci == sch.slot_last[s]))

                # ---- epilogue at slot end
                if ci == sch.slot_last[s]:
                    asb = aggsb.tile([P, k192], F32)
                    nc.scalar.activation(asb[:], aggr_tile[:, 0:k192],
                                         mybir.ActivationFunctionType.Copy)
                    ep = epip.tile([P, 512], F32)
                    at1 = atsb.tile([half_k, P], F16, tag="at1")
                    at2 = atsb.tile([half_k, P], F16, tag="at2")
                    nc.tensor.transpose(ep[0:half_k, 0:P], asb[:, 0:half_k],
                                        ident[:])
                    nc.scalar.activation(at1[:], ep[0:half_k, 0:P],
                                         mybir.ActivationFunctionType.Copy)
                    nc.tensor.transpose(ep[0:half_k, P:2 * P],
                                        asb[:, half_k:k192], ident[:])
                    nc.scalar.activation(at2[:], ep[0:half_k, P:2 * P],
                                         mybir.ActivationFunctionType.Copy)
                    nc.tensor.matmul(ep[:, 2 * P:3 * P], at1[:], wo1[:],
                                     start=True, stop=False)
                    nc.tensor.matmul(ep[:, 2 * P:3 * P], at2[:],
                                     wo2[:], start=False, stop=False)
                    nc.tensor.matmul(ep[:, 2 * P:3 * P], ones1[:],
                                     wo2b[:], start=False, stop=True)
                    osb = outsb.tile([P, cfg.c_out], F32)
                    nc.scalar.activation(osb[:], ep[:, 2 * P:3 * P],
                                         mybir.ActivationFunctionType.Tanh)
                    nc.sync.dma_start(
                        out=out_scr[li, s * P:(s + 1) * P, :], in_=osb[:])

        if reps > 1:
            with tc.For_i(0, reps, 1):
                for li, sch in enumerate(scheds):
                    emit_layer(li, sch)
        else:
            for li, sch in enumerate(scheds):
                emit_layer(li, sch)
    nc.finalize()
    return nc


# --------------------------------------------------------------------------
# entry point
# --------------------------------------------------------------------------

def make_in_maps(cfg, x, scheds):
    x = np.asarray(x, np.float32)
    if cfg.g128:
        xp = np.zeros((cfg.n_nodes, 2 * cfg.c_in), dtype=np.float16)
        xp[:, :cfg.c_in] = x.astype(np.float16)
        tlo = np.ascontiguousarray(xp[:cfg.half])
        thi = np.ascontiguousarray(xp[cfg.half:])
    else:
        tlo = np.ascontiguousarray(x[:cfg.half])
        thi = np.ascontiguousarray(x[cfg.half:])
    in_maps = []
    for k in range(cfg.n_cores):
        m = {"x_lo": tlo, "x_hi": thi}
        for li, sch in enumerate(scheds):
            c = sch.core[k]
            m[f"idx_lo{li}"] = c["idx_lo"]
            m[f"idx_hi{li}"] = c["idx_hi"]
            m[f"attr14_{li}"] = c["attr14"]
            m[f"oneh{li}"] = c["oneh"]
            m[f"W14_{li}"] = sch.W14
            m[f"Wout1_{li}"] = sch.Wout1
            m[f"Wout2_{li}"] = sch.Wout2
        in_maps.append(m)
    return in_maps


def _run(cfg, x, layers, reps=1):
    """layers: list of (edge_index, edge_attr, W_in, b_in, W_out, b_out)."""
    scheds = [prep_layer(cfg, *lay) for lay in layers]
    nc = build_program(cfg, scheds, reps=reps)
    in_maps = make_in_maps(cfg, x, scheds)

    from concourse.bass_utils import run_bass_kernel_spmd
    res = run_bass_kernel_spmd(nc, in_maps, list(range(cfg.n_cores)),
                               trace=globals().get("TRACE", False))

    n_layers = len(layers)
    out = np.zeros((cfg.n_nodes, n_layers * cfg.c_out), dtype=np.float32)
    for k in range(cfg.n_cores):
        scr = res.results[k]["out_scr"]
        for li, sch in enumerate(scheds):
            for s in range(cfg.n_slot):
                w = sch.win_of[k, s]
                if w < 0:
                    continue
                r0 = int(w) * P
                r1 = min(r0 + P, cfg.n_nodes)
                out[r0:r1, li * cfg.c_out:(li + 1) * cfg.c_out] = \
                    scr[li, s * P:s * P + (r1 - r0), :]
    return out, res


def kernel(x, edge_index0, edge_attr0, edge_index1, edge_attr1,
           W_in0, b_in0, W_out0, b_out0, W_in1, b_in1, W_out1, b_out1):
    x = np.asarray(x)
    cfg = Cfg(n_nodes=x.shape[0], c_in=x.shape[1])
    layers = [
        (np.asarray(edge_index0), np.asarray(edge_attr0),
         np.asarray(W_in0), np.asarray(b_in0),
         np.asarray(W_out0), np.asarray(b_out0)),
        (np.asarray(edge_index1), np.asarray(edge_attr1),
         np.asarray(W_in1), np.asarray(b_in1),
         np.asarray(W_out1), np.asarray(b_out1)),
    ]
    out, _ = _run(cfg, x, layers)
    return out



# revision 20
# speedup vs baseline: 1.4875x; 1.0464x over previous
"""Trainium2 Bass kernel for nn_CFDFVGCN (two SpatialGraphConv layers, concat).

Strategy (8 NeuronCores, SPMD single program):
  - Shard by DESTINATION node windows of 128 nodes. 391 windows are balanced
    across 8 cores x 49 slots; each core aggregates its own windows entirely
    locally (no collectives).
  - Per (core, window, src-half) the edges are sorted by dst and cut into
    128-edge chunks (padded so chunk counts per slot are identical across
    cores -> one shared program).
  - Per chunk on device:
      scaling = relu([attr|1] @ [W_in;b_in])  (PE matmul, 2 chunks per matmul)
      msg     = scaling * x[src] (broadcast over hid=3)  (DVE, bf16)
      aggr   += onehot(dst_local)^T @ msg    (PE matmul into PSUM window)
    x[src] rows are fetched with dma_gather (SWDGE custom gather) from HBM.
    onehot matrices are host-precomputed fp8 and streamed in as weights.
  - Per window epilogue: transpose aggr, out = tanh(aggr @ W_out + b_out),
    DMA to a slot-indexed scratch; host unscrambles slots -> global rows.
"""

import math
import sys
from contextlib import ExitStack

import numpy as np

for _p in ("/opt/trn_rl_repo",):
    if _p not in sys.path:
        sys.path.insert(0, _p)

import ml_dtypes  # noqa: E402

import concourse.bacc as bacc  # noqa: E402
import concourse.bass as bass  # noqa: E402
import concourse.mybir as mybir  # noqa: E402
import concourse.tile as tile  # noqa: E402
from concourse.masks import make_identity  # noqa: E402

P = 128
F32 = mybir.dt.float32
F16 = mybir.dt.float16
FP8 = mybir.dt.float8e4
I16 = mybir.dt.int16

FP8_NP = mybir.dt.np(FP8)
F16_NP = np.float16


class Cfg:
    def __init__(self, n_nodes, c_in=64, hid=3, c_out=128, attr=6, n_cores=8,
                 gb=24, pb=24, eg=4, eg2=12, scratch=65536, ablate=(),
                 nq=4, aggr_bufs=2, epi_bufs=2, spsum_bufs=2, g128=True,
                 sbufs=4):
        self.n_nodes = n_nodes
        self.c_in = c_in            # 64
        self.hid = hid              # 3
        self.c_out = c_out          # 128
        self.attr = attr            # 6
        self.n_cores = n_cores
        self.half = (n_nodes + 1) // 2      # src split point for int16 gather
        self.n_win = (n_nodes + P - 1) // P
        self.n_slot = (self.n_win + n_cores - 1) // n_cores
        self.gb = gb                # chunks per gather / onehot batch
        self.pb = pb                # scaling pairs per attr batch
        self.eg = eg                # chunks per PSUM scaling group (eg/2 banks)
        self.eg2 = eg2              # chunks per elementwise supergroup
        self.sbufs = sbufs          # bufs for scal/xs/msg pools
        assert eg2 % eg == 0
        self.k192 = hid * c_in      # 192
        self.scratch = scratch
        self.ablate = set(ablate)
        self.nq = nq
        self.g128 = g128
        self.aggr_bufs = aggr_bufs
        self.epi_bufs = epi_bufs
        self.spsum_bufs = spsum_bufs
        assert (eg // 2) * spsum_bufs + aggr_bufs + epi_bufs <= 8
        # feature permutation: device feature j' = h*64+c  <->  ref j = 3c+h
        c = np.arange(c_in)
        self.perm = np.concatenate([hid * c + h for h in range(hid)])  # [192]


# --------------------------------------------------------------------------
# host-side scheduling / data prep
# --------------------------------------------------------------------------

class LayerSched:
    """Per-layer, cross-core-uniform chunk schedule + per-core data arrays."""
    pass


def _assign_windows(cfg, win_edge_counts):
    """Balance windows across cores. Returns win_of[core][slot] (-1 = pad)."""
    n_win, n_cores, n_slot = cfg.n_win, cfg.n_cores, cfg.n_slot
    order = np.argsort(-win_edge_counts, kind="stable")
    totals = np.zeros(n_cores, dtype=np.int64)
    counts = np.zeros(n_cores, dtype=np.int64)
    win_of = -np.ones((n_cores, n_slot), dtype=np.int64)
    for w in order:
        k = min((kk for kk in range(n_cores) if counts[kk] < n_slot),
                key=lambda kk: totals[kk])
        win_of[k, counts[k]] = w
        counts[k] += 1
        totals[k] += win_edge_counts[w]
    # within each core, sort slots by edge count desc so slot s across cores
    # holds comparably-sized windows (minimizes per-slot max padding)
    for k in range(n_cores):
        cnt = np.where(win_of[k] >= 0, win_edge_counts[np.maximum(win_of[k], 0)], -1)
        win_of[k] = win_of[k][np.argsort(-cnt, kind="stable")]
    return win_of


def prep_layer(cfg, edge_index, edge_attr, W_in, b_in, W_out, b_out):
    """Build the uniform schedule and per-core device arrays for one layer."""
    src = np.asarray(edge_index[0], dtype=np.int64).astype(np.int32)
    dst = np.asarray(edge_index[1], dtype=np.int64).astype(np.int32)
    attr = np.asarray(edge_attr, dtype=np.float32)
    E = src.shape[0]
    n_cores, n_slot = cfg.n_cores, cfg.n_slot

    win = dst >> 7
    if "oldassign" in cfg.ablate:
        win_counts = np.bincount(win, minlength=cfg.n_win)
        win_of = _assign_windows(cfg, win_counts)
        _skip_deal = True
    else:
        _skip_deal = False
    n_winp = cfg.n_cores * cfg.n_slot
    half_all = (src >= cfg.half)
    cl = np.bincount(win[~half_all], minlength=n_winp)
    chh = np.bincount(win[half_all], minlength=n_winp)
    clc, chc = (cl + P - 1) // P, (chh + P - 1) // P
    # deal windows sorted by chunk signature: slot s gets ranks 8s..8s+7,
    # one per core -> per-slot max over cores is tight
    if not _skip_deal:
        order = sorted(range(n_winp), key=lambda w: (int(clc[w]), int(chc[w])))
        win_of = -np.ones((cfg.n_cores, cfg.n_slot), dtype=np.int64)
        for i, w in enumerate(order):
            if w < cfg.n_win:
                win_of[i % cfg.n_cores, i // cfg.n_cores] = w

    # per (core, slot, half): edge index lists sorted by dst
    # edge order: sort once globally by (win, srchalf, dst)
    half = (src >= cfg.half).astype(np.int64)
    okey = (win.astype(np.int64) << 33) | (half << 32) | dst.astype(np.int64)
    eorder = np.argsort(okey, kind="stable")
    # boundaries per (win, half)
    wh = win.astype(np.int64) * 2 + half
    wh_sorted = wh[eorder]
    grp_start = {}
    uniq, starts = np.unique(wh_sorted, return_index=True)
    ends = np.append(starts[1:], E)
    for u, s0, e0 in zip(uniq, starts, ends):
        grp_start[int(u)] = (int(s0), int(e0))

    def group_edges(w, h):
        r = grp_start.get(int(w) * 2 + int(h))
        if r is None:
            return eorder[0:0]
        return eorder[r[0]:r[1]]

    # chunk counts per (core, slot, half)
    nch = np.zeros((n_cores, n_slot, 2), dtype=np.int64)
    for k in range(n_cores):
        for s in range(n_slot):
            w = win_of[k, s]
            if w < 0:
                continue
            for h in range(2):
                cnt = len(group_edges(w, h))
                nch[k, s, h] = (cnt + P - 1) // P
    CH = nch.max(axis=0)  # [n_slot, 2] uniform chunk counts
    for s in range(n_slot):
        if CH[s].sum() == 0:
            CH[s, 0] = 1  # every slot needs >=1 chunk to init its PSUM bank

    # global chunk schedule: per slot, lo-run then hi-run
    sched_chunks = []       # (slot, half)
    for s in range(n_slot):
        sched_chunks += [(s, 0)] * int(CH[s, 0]) + [(s, 1)] * int(CH[s, 1])
    # pad to multiple of eg2 (and so of 2) with hi-chunks on the last slot
    while len(sched_chunks) % cfg.eg2 != 0:
        sched_chunks.append((n_slot - 1, 1))
        CH[n_slot - 1, 1] += 1
    NCH = len(sched_chunks)

    # stream positions
    stream_pos = []         # per chunk: (half, pos in that stream)
    cnt_lo = cnt_hi = 0
    for (s, h) in sched_chunks:
        if h == 0:
            stream_pos.append((0, cnt_lo)); cnt_lo += 1
        else:
            stream_pos.append((1, cnt_hi)); cnt_hi += 1
    NLO, NHI = cnt_lo, cnt_hi
    NBLO = (NLO + cfg.gb - 1) // cfg.gb if NLO else 0
    NBHI = (NHI + cfg.gb - 1) // cfg.gb if NHI else 0
    NPAIR = NCH // 2
    NB14 = (NPAIR + cfg.pb - 1) // cfg.pb

    # slot boundaries: first/last chunk index per slot
    slot_first = {}
    slot_last = {}
    for ci, (s, h) in enumerate(sched_chunks):
        if s not in slot_first:
            slot_first[s] = ci
        slot_last[s] = ci

    sch = LayerSched()
    sch.cfg = cfg
    sch.win_of = win_of
    sch.CH = CH
    sch.chunks = sched_chunks
    sch.NCH = NCH
    sch.stream_pos = stream_pos
    sch.NLO, sch.NHI, sch.NBLO, sch.NBHI = NLO, NHI, NBLO, NBHI
    sch.NPAIR, sch.NB14 = NPAIR, NB14
    sch.slot_first, sch.slot_last = slot_first, slot_last

    # ---- weights (shared across cores)
    k192 = cfg.k192
    W7 = np.concatenate([np.asarray(W_in, np.float32),
                         np.asarray(b_in, np.float32)[None, :]], axis=0)  # [7,192]
    W7r = W7[:, cfg.perm]                                                 # [7,192]
    na = cfg.attr + 1
    W14 = np.zeros((2 * na, 2 * k192), dtype=np.float32)
    W14[:na, :k192] = W7r
    W14[na:, k192:] = W7r
    Woutr = np.asarray(W_out, np.float32)[cfg.perm, :]                    # [192,128]
    Wout1 = np.ascontiguousarray(Woutr[: k192 // 2])                      # [96,128]
    Wout2 = np.concatenate([Woutr[k192 // 2:],
                            np.asarray(b_out, np.float32)[None, :]], 0)   # [97,128]
    sch.W14 = W14.astype(np.float16)
    sch.Wout1 = Wout1.astype(np.float16)
    sch.Wout2 = Wout2.astype(np.float16)

    # ---- per-core arrays
    sch.core = []
    iota128 = np.arange(P, dtype=np.int32)
    for k in range(n_cores):
        # flat per-chunk edge ids (-1 = pad)
        eids = -np.ones((NCH, P), dtype=np.int64)
        for s in range(n_slot):
            w = win_of[k, s]
            if w < 0:
                continue
            for h in range(2):
                g = group_edges(w, h)
                if len(g) == 0:
                    continue
                # chunk indices for this (s,h)
                base = slot_first[s] + (0 if h == 0 else int(CH[s, 0]))
                ncs = (len(g) + P - 1) // P
                for c in range(ncs):
                    eids[base + c, : len(g[c * P:(c + 1) * P])] = g[c * P:(c + 1) * P]
        valid = eids >= 0
        e_safe = np.maximum(eids, 0)

        srcs = np.where(valid, src[e_safe], 0)
        dsts = np.where(valid, dst[e_safe], 0)

        # gather indices per stream, wrapped [16, n/16] replicated to 128 parts
        def build_idx(nb, stream_h):
            arr = np.zeros((max(nb, 1), P, cfg.gb * 8), dtype=np.int16)
            if nb == 0:
                return arr
            flat = np.zeros(nb * cfg.gb * P, dtype=np.int16)
            pos = 0
            for ci in range(NCH):
                hh, pp = stream_pos[ci]
                if hh != stream_h:
                    continue
                v = srcs[ci].astype(np.int64) - (cfg.half if stream_h else 0)
                v = np.where(valid[ci], v, 0)
                flat[pp * P:(pp + 1) * P] = v.astype(np.int16)
                pos += 1
            wrapped = flat.reshape(nb, cfg.gb * 8, 16).transpose(0, 2, 1)  # [nb,16,gb*8]
            return np.ascontiguousarray(
                np.tile(wrapped, (1, 8, 1)).astype(np.int16))              # [nb,128,gb*8]

        idx_lo = build_idx(NBLO, 0)
        idx_hi = build_idx(NBHI, 1)

        # attr14: [NB14, 14, pb*128]
        attr14 = np.zeros((NB14, 2 * na, cfg.pb * P), dtype=np.float16)
        a7 = np.zeros((NCH, na, P), dtype=np.float32)
        a7[:, :cfg.attr, :] = np.where(valid[:, None, :],
                                       attr[e_safe].transpose(0, 2, 1), 0.0)
        a7[:, cfg.attr, :] = valid.astype(np.float32)
        for pr in range(NPAIR):
            b14, off = divmod(pr, cfg.pb)
            attr14[b14, :na, off * P:(off + 1) * P] = a7[2 * pr]
            attr14[b14, na:, off * P:(off + 1) * P] = a7[2 * pr + 1]

        # onehot fp8: [128, NCH*128] partition-major (batched by gb chunks)
        slot_ids = np.array([s for (s, h) in sched_chunks], dtype=np.int64)
        dloc = dsts - (win_of[k][slot_ids][:, None] * P)
        dloc = np.where(valid, dloc, -1)
        oh = (dloc[:, :, None] == iota128[None, None, :])                 # [NCH,128,128]
        oh8 = np.where(oh, np.uint8(0x38), np.uint8(0)).transpose(1, 0, 2)
        oh8 = oh8.reshape(P, NCH * P)
        nbo = (NCH + cfg.gb - 1) // cfg.gb
        if nbo * cfg.gb * P > NCH * P:  # pad to whole onehot batches
            pad = np.zeros((P, nbo * cfg.gb * P - NCH * P), dtype=np.uint8)
            oh8 = np.concatenate([oh8, pad], axis=1)
        oh8 = np.ascontiguousarray(oh8).view(FP8_NP)

        core = {"idx_lo": idx_lo, "idx_hi": idx_hi, "attr14": attr14, "oneh": oh8}
        sch.core.append(core)
    return sch


def _dma_gather_any(g, out_ap, in_ap, idxs_ap, num_idxs, num_idxs_reg,
                    elem_size, elem_step, single_packet=False, queue_num=0):
    """dma_gather without the elem_size%256 restriction (row stride must
    still be a 256B multiple; gathers the first elem_size elems per row)."""
    stride_bytes = elem_step * mybir.dt.size(in_ap.dtype)
    assert stride_bytes % 256 == 0 and stride_bytes // 256 < 256
    _in_ap = g.lower_ap_dma(in_ap, for_custom_bir_dma=True)
    _idxs_ap = g.lower_ap(idxs_ap)
    _out_ap = g.lower_ap(out_ap)
    return g.add_instruction(mybir.InstDMAGatherAnt(
        name=g.bass.get_next_instruction_name(),
        ins=[*_in_ap, _idxs_ap, g.lower_val_access(g.to_reg(num_idxs_reg))],
        outs=[_out_ap],
        transpose=False, num_idxs=num_idxs, elem_size=elem_size,
        stride_bytes_256=stride_bytes // 256, gen_mode=0,
        single_packet=single_packet, queue_num=queue_num,
        sbuf_tokens_per_rank=0, sbuf_free_dim_per_rank=0,
        sbuf_free_dim_pad_per_rank=0, sbuf_byte_offset=0))


# --------------------------------------------------------------------------
# device program
# --------------------------------------------------------------------------

def build_program(cfg, scheds, reps=1):
    """One shared SPMD program for all cores. scheds = [layer0, layer1]."""
    nc = bacc.Bacc("TRN2", target_bir_lowering=False, debug=False,
               dynamic_dma_scratch_size=cfg.scratch,
               num_swdge_queues=cfg.nq)
    n_layers = len(scheds)
    na = cfg.attr + 1
    k192, half_k = cfg.k192, cfg.k192 // 2

    # ---- DRAM tensors (identical shapes across cores)
    if cfg.g128:
        x_lo = nc.dram_tensor("x_lo", [cfg.half, 2 * cfg.c_in], F16,
                              kind="ExternalInput")
        x_hi = nc.dram_tensor("x_hi", [cfg.n_nodes - cfg.half, 2 * cfg.c_in],
                              F16, kind="ExternalInput")
    else:
        x_lo = nc.dram_tensor("x_lo", [cfg.half, cfg.c_in], F32,
                              kind="ExternalInput")
        x_hi = nc.dram_tensor("x_hi", [cfg.n_nodes - cfg.half, cfg.c_in], F32,
                              kind="ExternalInput")
    dins = []
    for li, sch in enumerate(scheds):
        d = {}
        d["idx_lo"] = nc.dram_tensor(f"idx_lo{li}", list(sch.core[0]["idx_lo"].shape),
                                     I16, kind="ExternalInput")
        d["idx_hi"] = nc.dram_tensor(f"idx_hi{li}", list(sch.core[0]["idx_hi"].shape),
                                     I16, kind="ExternalInput")
        d["attr14"] = nc.dram_tensor(f"attr14_{li}", list(sch.core[0]["attr14"].shape),
                                     F16, kind="ExternalInput")
        d["oneh"] = nc.dram_tensor(f"oneh{li}", list(sch.core[0]["oneh"].shape),
                                   FP8, kind="ExternalInput")
        d["W14"] = nc.dram_tensor(f"W14_{li}", list(sch.W14.shape), F16,
                                  kind="ExternalInput")
        d["Wout1"] = nc.dram_tensor(f"Wout1_{li}", list(sch.Wout1.shape), F16,
                                    kind="ExternalInput")
        d["Wout2"] = nc.dram_tensor(f"Wout2_{li}", list(sch.Wout2.shape), F16,
                                    kind="ExternalInput")
        dins.append(d)
    out_scr = nc.dram_tensor("out_scr", [n_layers, cfg.n_slot * P, cfg.c_out], F32,
                             kind="ExternalOutput")

    with tile.TileContext(nc) as tc, ExitStack() as ctx:
        const = ctx.enter_context(tc.tile_pool(name="const", bufs=1))
        idxp = (None if "nogather" in cfg.ablate else
                ctx.enter_context(tc.tile_pool(name="idx", bufs=2)))
        xjp = [ctx.enter_context(tc.tile_pool(name=f"xj{h}", bufs=2)) for h in range(2)]
        attrp = ctx.enter_context(tc.tile_pool(name="attr", bufs=2))
        onehp = ctx.enter_context(tc.tile_pool(name="oneh", bufs=2))
        spsum = ctx.enter_context(tc.tile_pool(name="spsum", bufs=cfg.spsum_bufs, space="PSUM"))
        aggrp = ctx.enter_context(tc.tile_pool(name="aggr", bufs=cfg.aggr_bufs, space="PSUM"))
        epip = ctx.enter_context(tc.tile_pool(name="epip", bufs=cfg.epi_bufs, space="PSUM"))
        scalp = ctx.enter_context(tc.tile_pool(name="scal", bufs=cfg.sbufs))
        msgp = ctx.enter_context(tc.tile_pool(name="msg", bufs=cfg.sbufs))
        aggsb = ctx.enter_context(tc.tile_pool(name="aggsb", bufs=2))
        atsb = ctx.enter_context(tc.tile_pool(name="atsb", bufs=2))
        outsb = ctx.enter_context(tc.tile_pool(name="outsb", bufs=2))

        ident = const.tile([P, P], F32)
        make_identity(nc, ident[:])
        ones1 = const.tile([1, P], F16)
        nc.vector.memset(ones1[:], 1.0)

        wtiles = []
        for li, sch in enumerate(scheds):
            d = dins[li]
            w14 = const.tile([2 * na, 2 * k192], F16, tag=f"w14_{li}")
            nc.sync.dma_start(out=w14[:], in_=d["W14"][:])
            wo1 = const.tile([half_k, cfg.c_out], F16, tag=f"wo1_{li}")
            nc.sync.dma_start(out=wo1[:], in_=d["Wout1"][:])
            wo2 = const.tile([half_k, cfg.c_out], F16, tag=f"wo2_{li}")
            nc.sync.dma_start(out=wo2[:], in_=d["Wout2"][0:half_k])
            wo2b = const.tile([1, cfg.c_out], F16, tag=f"wo2b_{li}")
            nc.sync.dma_start(out=wo2b[:], in_=d["Wout2"][half_k:half_k + 1])
            wtiles.append((w14, wo1, wo2, wo2b))

        def emit_layer(li, sch):
            d = dins[li]
            w14, wo1, wo2, wo2b = wtiles[li]

            xj_tile_of = [{}, {}]        # stream -> {batch index: tile}
            oneh_tile = None
            attr_tile = None
            attr_b14 = -1
            sp_tile = None               # scaling psum group tile
            scal_tile = xs_tile = msg_tile = None
            aggr_tile = None

            def ensure_gather(stream, bi):
                if bi in xj_tile_of[stream]:
                    return
                xt = xjp[stream].tile([P, cfg.gb, cfg.c_in],
                                      F16 if cfg.g128 else F32)
                if "nogather" in cfg.ablate:
                    nc.gpsimd.memset(xt[:, 0:1, :], 0.25)
                else:
                    it = idxp.tile([P, cfg.gb * 8], I16)
                    src_dram = d["idx_lo"] if stream == 0 else d["idx_hi"]
                    nc.sync.dma_start(out=it[:], in_=src_dram[bi])
                    table = x_lo if stream == 0 else x_hi
                    if cfg.g128:
                        _dma_gather_any(
                            nc.gpsimd, out_ap=xt[:],
                            in_ap=table[:, 0:cfg.c_in], idxs_ap=it[:],
                            num_idxs=cfg.gb * P, num_idxs_reg=cfg.gb * P,
                            elem_size=cfg.c_in, elem_step=2 * cfg.c_in,
                            single_packet=False,
                            queue_num=(stream * 7 + bi) % cfg.nq)
                    else:
                        nc.gpsimd.dma_gather(
                            out_ap=xt[:], in_ap=table[:], idxs_ap=it[:],
                            num_idxs=cfg.gb * P, num_idxs_reg=cfg.gb * P,
                            elem_size=cfg.c_in, single_packet=False,
                            queue_num=(stream * 7 + bi) % cfg.nq)
                xj_tile_of[stream][bi] = xt

            for ci in range(sch.NCH):
                s, h = sch.chunks[ci]

                # ---- supergroup head: gathers, scaling, relu, stage, mul
                if ci % cfg.eg2 == 0:
                    for cj in range(ci, ci + cfg.eg2):
                        st_j, pos_j = sch.stream_pos[cj]
                        ensure_gather(st_j, pos_j // cfg.gb)
                    scal_tile = scalp.tile([P, cfg.eg2, k192], F16)
                    for g0 in range(0, cfg.eg2, cfg.eg):
                        sp_tile = spsum.tile([P, cfg.eg // 2, 512], F32)
                        for pj in range(cfg.eg // 2):
                            pr = (ci + g0) // 2 + pj
                            b14, poff = divmod(pr, cfg.pb)
                            if b14 != attr_b14:
                                attr_tile = attrp.tile([2 * na, cfg.pb * P],
                                                       F16)
                                nc.sync.dma_start(out=attr_tile[:],
                                                  in_=d["attr14"][b14])
                                attr_b14 = b14
                            if "nosc" in cfg.ablate:
                                nc.vector.memset(sp_tile[:, pj, 0:8], 0.0)
                            else:
                                nc.tensor.matmul(
                                    sp_tile[:, pj, 0:2 * k192],
                                    attr_tile[:, poff * P:(poff + 1) * P],
                                    w14[:],
                                    start=True, stop=True)
                        if "noelem" in cfg.ablate:
                            nc.gpsimd.memset(
                                scal_tile[:, g0:g0 + 1, 0:8], 0.25)
                        else:
                            nc.scalar.activation(
                                scal_tile[:, g0:g0 + cfg.eg, :],
                                sp_tile[:, :, 0:2 * k192],
                                mybir.ActivationFunctionType.Relu)
                    # msg = scal * xj, read directly from the gather tiles per
                    # contiguous run. tensor_tensor only runs in 1-port DVE
                    # mode — 2-port DVE ops (copy/cast/memset) fully block
                    # SWDGE descriptor generation and starve the gathers.
                    msg_tile = msgp.tile([P, cfg.eg2, k192], F16)
                    if "noelem" in cfg.ablate:
                        nc.gpsimd.memset(msg_tile[:, 0:1, 0:8], 0.25)
                    j = 0
                    while j < cfg.eg2 and "noelem" not in cfg.ablate:
                        st_j, pos_j = sch.stream_pos[ci + j]
                        bi_j, off_j = divmod(pos_j, cfg.gb)
                        run = 1
                        while (j + run < cfg.eg2):
                            st_n, pos_n = sch.stream_pos[ci + j + run]
                            if st_n != st_j or pos_n != pos_j + run:
                                break
                            if divmod(pos_n, cfg.gb)[0] != bi_j:
                                break
                            run += 1
                        xsl = xj_tile_of[st_j][bi_j][:, off_j:off_j + run, :]
                        for hh in range(cfg.hid):
                            o0 = hh * cfg.c_in
                            nc.vector.tensor_tensor(
                                out=msg_tile[:, j:j + run, o0:o0 + cfg.c_in],
                                in0=scal_tile[:, j:j + run, o0:o0 + cfg.c_in],
                                in1=xsl,
                                op=mybir.AluOpType.mult)
                        j += run

                # ---- onehot batch load
                obi, ooff = divmod(ci, cfg.gb)
                if ooff == 0 and "noscatter" not in cfg.ablate:
                    oneh_tile = onehp.tile([P, cfg.gb * P], FP8)
                    # scalar (ACT) HWDGE ring — keeps the big onehot stream
                    # off the SP ring that serves attr/idx/out
                    nc.scalar.dma_start(
                        out=oneh_tile[:],
                        in_=d["oneh"][:, obi * cfg.gb * P:(obi + 1) * cfg.gb * P])

                eoff = ci % cfg.eg2
                # ---- scatter matmul into the slot's aggr window
                if "noscatter" in cfg.ablate:
                    continue
                if ci == sch.slot_first[s]:
                    aggr_tile = aggrp.tile([P, 512], F32)
                nc.tensor.matmul(
                    aggr_tile[:, 0:k192],
                    oneh_tile[:, ooff * P:(ooff + 1) * P],
                    msg_tile[:, eoff, :],
                    start=(ci == sch.slot_first[s]),
                    stop=(ci == sch.slot_last[s]))

                # ---- epilogue at slot end
                if ci == sch.slot_last[s]:
                    asb = aggsb.tile([P, k192], F32)
                    nc.scalar.activation(asb[:], aggr_tile[:, 0:k192],
                                         mybir.ActivationFunctionType.Copy)
                    ep = epip.tile([P, 512], F32)
                    at1 = atsb.tile([half_k, P], F16, tag="at1")
                    at2 = atsb.tile([half_k, P], F16, tag="at2")
                    nc.tensor.transpose(ep[0:half_k, 0:P], asb[:, 0:half_k],
                                        ident[:])
                    nc.scalar.activation(at1[:], ep[0:half_k, 0:P],
                                         mybir.ActivationFunctionType.Copy)
                    nc.tensor.transpose(ep[0:half_k, P:2 * P],
                                        asb[:, half_k:k192], ident[:])
                    nc.scalar.activation(at2[:], ep[0:half_k, P:2 * P],
                                         mybir.ActivationFunctionType.Copy)
                    nc.tensor.matmul(ep[:, 2 * P:3 * P], at1[:], wo1[:],
                                     start=True, stop=False)
                    nc.tensor.matmul(ep[:, 2 * P:3 * P], at2[:],
                                     wo2[:], start=False, stop=False)
                    nc.tensor.matmul(ep[:, 2 * P:3 * P], ones1[:],
                                     wo2b[:], start=False, stop=True)
                    osb = outsb.tile([P, cfg.c_out], F32)
                    nc.scalar.activation(osb[:], ep[:, 2 * P:3 * P],
                                         mybir.ActivationFunctionType.Tanh)
                    nc.sync.dma_start(
                        out=out_scr[li, s * P:(s + 1) * P, :], in_=osb[:])

        if reps > 1:
            with tc.For_i(0, reps, 1):
                for li, sch in enumerate(scheds):
                    emit_layer(li, sch)
        else:
            for li, sch in enumerate(scheds):
                emit_layer(li, sch)
    nc.finalize()
    return nc


# --------------------------------------------------------------------------
# entry point
# --------------------------------------------------------------------------

def make_in_maps(cfg, x, scheds):
    x = np.asarray(x, np.float32)
    if cfg.g128:
        xp = np.zeros((cfg.n_nodes, 2 * cfg.c_in), dtype=np.float16)
        xp[:, :cfg.c_in] = x.astype(np.float16)
        tlo = np.ascontiguousarray(xp[:cfg.half])
        thi = np.ascontiguousarray(xp[cfg.half:])
    else:
        tlo = np.ascontiguousarray(x[:cfg.half])
        thi = np.ascontiguousarray(x[cfg.half:])
    in_maps = []
    for k in range(cfg.n_cores):
        m = {"x_lo": tlo, "x_hi": thi}
        for li, sch in enumerate(scheds):
            c = sch.core[k]
            m[f"idx_lo{li}"] = c["idx_lo"]
            m[f"idx_hi{li}"] = c["idx_hi"]
            m[f"attr14_{li}"] = c["attr14"]
            m[f"oneh{li}"] = c["oneh"]
            m[f"W14_{li}"] = sch.W14
            m[f"Wout1_{li}"] = sch.Wout1
            m[f"Wout2_{li}"] = sch.Wout2
        in_maps.append(m)
    return in_maps


def _run(cfg, x, layers, reps=1):
    """layers: list of (edge_index, edge_attr, W_in, b_in, W_out, b_out)."""
    scheds = [prep_layer(cfg, *lay) for lay in layers]
    nc = build_program(cfg, scheds, reps=reps)
    in_maps = make_in_maps(cfg, x, scheds)

    from concourse.bass_utils import run_bass_kernel_spmd
    res = run_bass_kernel_spmd(nc, in_maps, list(range(cfg.n_cores)),
                               trace=globals().get("TRACE", False))

    n_layers = len(layers)
    out = np.zeros((cfg.n_nodes, n_layers * cfg.c_out), dtype=np.float32)
    for k in range(cfg.n_cores):
        scr = res.results[k]["out_scr"]
        for li, sch in enumerate(scheds):
            for s in range(cfg.n_slot):
                w = sch.win_of[k, s]
                if w < 0:
                    continue
                r0 = int(w) * P
                r1 = min(r0 + P, cfg.n_nodes)
                out[r0:r1, li * cfg.c_out:(li + 1) * cfg.c_out] = \
                    scr[li, s * P:s * P + (r1 - r0), :]
    return out, res


def kernel(x, edge_index0, edge_attr0, edge_index1, edge_attr1,
           W_in0, b_in0, W_out0, b_out0, W_in1, b_in1, W_out1, b_out1):
    x = np.asarray(x)
    cfg = Cfg(n_nodes=x.shape[0], c_in=x.shape[1])
    layers = [
        (np.asarray(edge_index0), np.asarray(edge_attr0),
         np.asarray(W_in0), np.asarray(b_in0),
         np.asarray(W_out0), np.asarray(b_out0)),
        (np.asarray(edge_index1), np.asarray(edge_attr1),
         np.asarray(W_in1), np.asarray(b_in1),
         np.asarray(W_out1), np.asarray(b_out1)),
    ]
    out, _ = _run(cfg, x, layers)
    return out

